# revision 1
# baseline (speedup 1.0000x reference)
"""Trainium2 Bass/Tile kernel for nn_Decoder (GRU decoder with teacher forcing).

Math (per reference):
  zx  = [enc_h_feat, z]                    (B, 1056)
  h0  = zx @ W_dh.T + b_dh                 (B, 128)
  a0  = last_obs @ W_vel.T + b_vel         (B, 2)
  rel = (sg - last_obs[:, :2]) / dt        (B, 2)
  a_t = a0 if t==0 else fut_traj[t-1,:,2:4]
  x_t = [zx, a_t, rel]  -> GRUCell(x_t, h) -> mu_t, std_t

Device strategy (8 cores, batch-sharded, 2048 rows/core):
  - Feature-on-partition, batch-on-free layout; free chunks of 512.
  - Setup: one fp32r matmul  [gi_r|gi_z|gi_n|h0](512 rows) = W_big.T @ XT
    with K=1065 host-packed rows [zxT; sgT; loT; ones].  The rel term and all
    input-side biases are folded into W_big on the host (weight-only math).
  - Per step (bf16 matmuls, fp32 PSUM):
      psum_rz = Whh_{r,z} @ h  (+ K=3 matmul: a0,a1,ones rows -> Wa + b_hh)
      rz      = sigmoid(psum_rz)                      [ScalarE, direct from PSUM]
      q       = (psum_hn + b_hh_n) * r                [DVE scalar_tensor_tensor]
      psum_gin= I@gi_n + K2@a + I@q                   [PE identity-adds]
      n       = tanh(psum_gin)                        [ScalarE]
      d = h - n [GPSIMD], e = z*d [GPSIMD], h' = n + e [DVE]
      head: per-step matmul with a scattered-column lhsT variant accumulates
      mu/std pre-activations for ALL steps into 4 persistent PSUM tiles
      (one per batch chunk); one DVE copy per chunk at the end.
  - End: mu = Identity(H_mu + b_mu), std = Exp(0.5*H_std + 0.5*b_std)
    (std = sqrt(exp(x)) = exp(x/2), so no sqrt table is needed).
  - Recurrence runs in float32r (TF32-class matmul dtype, full rate for
    N>=256): ~30x lower error than bf16 at equal PE cost.
Host does only sharding/transposes/weight packing (a0 is a (B,6)@(6,2)
matmul on host, ~0.4 MFLOP, negligible vs the 52 GFLOP kernel).
"""

import numpy as np
import ml_dtypes

import concourse.bass as bass
import concourse.mybir as mybir
import concourse.tile as tile
from concourse import bacc
from concourse.bass_utils import run_bass_kernel_spmd

F32 = mybir.dt.float32
F32R = mybir.dt.float32r
BF16 = mybir.dt.bfloat16
AF = mybir.ActivationFunctionType
OP = mybir.AluOpType

B, T, MLP, ZD, H, NS, NP = 16384, 24, 1024, 32, 128, 6, 2
NCORES = 8
BC = B // NCORES            # 2048 rows per core
F = 512                     # free-dim chunk
NF = BC // F                # 4 chunks
KIN = MLP + ZD + NP + NS + 1  # 1065 = zx(1056) + sg(2) + lo(6) + ones(1)
NKC = (KIN + 127) // 128    # 9 K-chunks (8x128 + 41)
DT_CONST = 0.4 * 12


PRECISE = True   # fp32r recurrence (h, gi, q, a3, weights); gates/d/e fp32
INLINE_HEADS = True  # accumulate head matmuls in-loop in persistent PSUM
                     # (saves the 48MB h-store DRAM round-trip)
SMALLK_PACK = True   # stage a/bias rows at partitions {0,32,64} so the three
                     # K<=3 matmuls run as concurrent PE row-tiles


def build_nc(debug=False, t_steps=T):
    RD = F32R if PRECISE else BF16      # matmul-operand dtype for recurrence
    GD = F32 if PRECISE else BF16       # gate/elementwise dtype
    nc = bacc.Bacc("TRN2", target_bir_lowering=False, debug=debug)

    # ---- DRAM I/O ----
    xt_d = nc.dram_tensor("xt", [KIN, BC], F32, kind="ExternalInput").ap()
    wbig_d = nc.dram_tensor("wbig", [KIN, 512], F32, kind="ExternalInput").ap()
    WDT = F32 if PRECISE else BF16
    a3_d = nc.dram_tensor("a3", [t_steps, 3, BC], WDT, kind="ExternalInput").ap()
    whht_d = nc.dram_tensor("whht", [H, 3 * H], WDT, kind="ExternalInput").ap()
    k3rz_d = nc.dram_tensor("k3rz", [3, 2 * H], WDT, kind="ExternalInput").ap()
    k2n_d = nc.dram_tensor("k2n", [2, H], WDT, kind="ExternalInput").ap()
    k3p_d = nc.dram_tensor("k3p", [66, H], WDT, kind="ExternalInput").ap()
    _std_off = ((2 * t_steps + 31) // 32) * 32
    _m_head = _std_off + 2 * t_steps
    wmsx_d = nc.dram_tensor("wmsx", [H, t_steps * _m_head], WDT,
                            kind="ExternalInput").ap()
    id_d = nc.dram_tensor("ident128", [H, H], F32, kind="ExternalInput").ap()
    bhhn_d = nc.dram_tensor("bhhn", [H, 1], F32, kind="ExternalInput").ap()
    bmu_d = nc.dram_tensor("bmu48", [2 * t_steps, 1], F32, kind="ExternalInput").ap()
    bstd_d = nc.dram_tensor("bstd48", [2 * t_steps, 1], F32, kind="ExternalInput").ap()
    omu_d = nc.dram_tensor("omu", [2 * t_steps, BC], F32, kind="ExternalOutput").ap()
    ostd_d = nc.dram_tensor("ostd", [2 * t_steps, BC], F32, kind="ExternalOutput").ap()

    with tile.TileContext(nc) as tc:
        with tc.tile_pool(name="persist", bufs=1) as pp:
            # persistent SBUF state
            gi_r = pp.tile([H, BC], RD)
            gi_z = pp.tile([H, BC], RD)
            gi_n = pp.tile([H, BC], RD)
            hA = pp.tile([H, BC], RD)
            hB = pp.tile([H, BC], RD)
            # head accumulator rows: [mu0 xT | mu1 xT | pad | std0 xT | std1 xT]
            # (std block 32-aligned: compute APs need base partition % 32 == 0)
            std_off = ((2 * t_steps + 31) // 32) * 32
            m_head = std_off + 2 * t_steps
            hh = pp.tile([m_head, BC], F32, name="headacc")
            whht_t = pp.tile([H, 3 * H], RD)
            k3rz_t = pp.tile([3, 2 * H], RD)
            k2n_t = pp.tile([2, H], RD)
            k3p_t = pp.tile([66, H], RD)
            wmsx_t = pp.tile([H, t_steps * m_head], RD)
            bhhn_t = pp.tile([H, 1], F32)
            bmu_t = pp.tile([2 * t_steps, 1], F32)
            bstd_t = pp.tile([2 * t_steps, 1], F32)
            ident = pp.tile([H, H], RD)

            nc.sync.dma_start(whht_t[:], whht_d.bitcast(RD))
            nc.sync.dma_start(k3rz_t[:], k3rz_d.bitcast(RD))
            nc.sync.dma_start(k2n_t[:], k2n_d.bitcast(RD))
            nc.sync.dma_start(k3p_t[:], k3p_d.bitcast(RD))
            nc.sync.dma_start(wmsx_t[:], wmsx_d.bitcast(RD))
            nc.sync.dma_start(bhhn_t[:], bhhn_d)
            nc.sync.dma_start(bmu_t[:], bmu_d)
            nc.sync.dma_start(bstd_t[:], bstd_d)
            nc.sync.dma_start(ident[:], id_d.bitcast(RD) if PRECISE else id_d.bitcast(BF16))

            gi_dst = [gi_r, gi_z, gi_n, None]

            # ---- setup: [gi | h0] = W_big.T @ XT  (fp32r) ----
            with tc.tile_pool(name="xtp", bufs=1) as xtp, \
                 tc.tile_pool(name="wp", bufs=1) as wp, \
                 tc.tile_pool(name="sps", bufs=4, space="PSUM") as sps:
                # column-chunked xt loads: the first matmul group only
                # needs batch-chunk 0 of every K-chunk (2.2MB, not 9MB)
                xt_tiles = {}
                for nci in range(NF):
                    for k in range(NKC):
                        kc = min(128, KIN - 128 * k)
                        xt_k = xtp.tile([kc, F], F32R, name=f"xt{k}_{nci}",
                                        tag=f"xt{k}_{nci}")
                        _eng = [nc.sync, nc.scalar, nc.gpsimd][(nci * NKC + k) % 3]
                        _eng.dma_start(
                            xt_k[:],
                            xt_d[128 * k:128 * k + kc,
                                 nci * F:(nci + 1) * F].bitcast(F32R))
                        xt_tiles[(k, nci)] = xt_k
                w_tiles = []
                for m in range(4):
                    row = []
                    for k in range(NKC):
                        kc = min(128, KIN - 128 * k)
                        w_km = wp.tile([kc, 128], F32R, name=f"w{m}_{k}", tag=f"w{m}_{k}")
                        _eng = [nc.sync, nc.scalar, nc.gpsimd][(k + m) % 3]
                        _eng.dma_start(
                            w_km[:],
                            wbig_d[128 * k:128 * k + kc, 128 * m:128 * (m + 1)].bitcast(F32R),
                        )
                        row.append(w_km)
                    w_tiles.append(row)

                for m in range(4):
                    for nci in range(NF):
                        ps = sps.tile([128, F], F32, name="setps", tag="setps")
                        for k in range(NKC):
                            nc.tensor.matmul(
                                ps[:], w_tiles[m][k][:],
                                xt_tiles[(k, nci)][:],
                                start=(k == 0), stop=(k == NKC - 1),
                            )
                        dst = gi_dst[m] if m < 3 else hA
                        nc.vector.tensor_copy(dst[:, nci * F:(nci + 1) * F], ps[:])

            # ---- recurrence ----
            PRZ_BUFS = 1 if INLINE_HEADS else 2
            with tc.tile_pool(name="hdram", bufs=1, space="DRAM") as hdram, \
                 tc.tile_pool(name="a3p", bufs=3) as a3p, \
                 tc.tile_pool(name="gp", bufs=3) as gp, \
                 tc.tile_pool(name="prz", bufs=PRZ_BUFS, space="PSUM") as prz, \
                 tc.tile_pool(name="phn", bufs=1 if INLINE_HEADS else 2, space="PSUM") as phn, \
                 tc.tile_pool(name="pgin", bufs=1 if INLINE_HEADS else 2, space="PSUM") as pgin, \
                 tc.tile_pool(name="phd", bufs=1, space="PSUM") as phd:
                if INLINE_HEADS:
                    psum_hd = [
                        phd.tile([m_head, F], F32, name=f"pshd{c}", tag=f"pshd{c}")
                        for c in range(NF)
                    ]
                    hst = None
                else:
                    hst = hdram.tile([t_steps, H, BC], F32 if PRECISE else BF16,
                                     name="hst")
                for t in range(t_steps):
                    hcur = hA if t % 2 == 0 else hB
                    hnxt = hB if t % 2 == 0 else hA
                    if SMALLK_PACK:
                        a3_t = a3p.tile([66, BC], RD, name="a3t", tag="a3t")
                        _e = [nc.sync, nc.scalar, nc.gpsimd][t % 3]
                        _e.dma_start(a3_t[0:3, :], a3_d[t].bitcast(RD))
                        _e.dma_start(a3_t[32:35, :], a3_d[t].bitcast(RD))
                        _e.dma_start(a3_t[64:66, :], a3_d[t][0:2].bitcast(RD))
                    else:
                        a3_t = a3p.tile([3, BC], RD, name="a3t", tag="a3t")
                        [nc.sync, nc.scalar, nc.gpsimd][t % 3].dma_start(
                            a3_t[:], a3_d[t].bitcast(RD))
                    for c in range(NF):
                        cs = slice(c * F, (c + 1) * F)
                        hs = hcur[:, cs]
                        psum_rz = prz.tile([128, 2 * F], F32, name="psrz", tag="psrz")
                        # same-weight matmuls grouped: whh pair, K3 pair
                        # (concurrent row-tiles), identity pair -- minimizes
                        # LDWEIGHTS churn on the PE weight buffers
                        nc.tensor.matmul(psum_rz[:, 0:F], whht_t[:, 0:H], hs,
                                         start=True, stop=False)
                        nc.tensor.matmul(psum_rz[:, F:2 * F], whht_t[:, H:2 * H], hs,
                                         start=True, stop=False)
                        if SMALLK_PACK:
                            nc.tensor.matmul(psum_rz[:, 0:F], k3p_t[0:3, :],
                                             a3_t[0:3, cs], start=False, stop=False)
                            nc.tensor.matmul(psum_rz[:, F:2 * F], k3p_t[32:35, :],
                                             a3_t[32:35, cs], start=False, stop=False)
                        else:
                            nc.tensor.matmul(psum_rz[:, 0:F], k3rz_t[:, 0:H],
                                             a3_t[:, cs], start=False, stop=False)
                            nc.tensor.matmul(psum_rz[:, F:2 * F], k3rz_t[:, H:2 * H],
                                             a3_t[:, cs], start=False, stop=False)
                        nc.tensor.matmul(psum_rz[:, 0:F], ident[:], gi_r[:, cs],
                                         start=False, stop=True)
                        nc.tensor.matmul(psum_rz[:, F:2 * F], ident[:], gi_z[:, cs],
                                         start=False, stop=True)
                        psum_hn = phn.tile([128, F], F32, name="pshn", tag="pshn")
                        nc.tensor.matmul(psum_hn[:], whht_t[:, 2 * H:3 * H], hs,
                                         start=True, stop=True)
                        rz = gp.tile([128, 2 * F], GD, name="rz", tag="rz")
                        nc.scalar.activation(rz[:], psum_rz[:], AF.Sigmoid)
                        q = gp.tile([128, F], RD, name="q", tag="q")
                        nc.vector.scalar_tensor_tensor(
                            q[:], psum_hn[:], bhhn_t[:], rz[:, 0:F],
                            op0=OP.add, op1=OP.mult,
                        )
                        psum_gin = pgin.tile([128, F], F32, name="psgin", tag="psgin")
                        nc.tensor.matmul(psum_gin[:], ident[:], gi_n[:, cs],
                                         start=True, stop=False)
                        if SMALLK_PACK:
                            nc.tensor.matmul(psum_gin[:], k3p_t[64:66, :],
                                             a3_t[64:66, cs], start=False, stop=False)
                        else:
                            nc.tensor.matmul(psum_gin[:], k2n_t[:], a3_t[0:2, cs],
                                             start=False, stop=False)
                        nc.tensor.matmul(psum_gin[:], ident[:], q[:],
                                         start=False, stop=True)
                        nt = gp.tile([128, F], GD, name="nt", tag="nt")
                        nc.scalar.activation(nt[:], psum_gin[:], AF.Tanh)
                        d = gp.tile([128, F], GD, name="d", tag="d")
                        nc.gpsimd.tensor_tensor(d[:], hs, nt[:], op=OP.subtract)
                        e = gp.tile([128, F], GD, name="e", tag="e")
                        nc.gpsimd.tensor_tensor(e[:], rz[:, F:2 * F], d[:], op=OP.mult)
                        nc.vector.tensor_tensor(hnxt[:, cs], nt[:], e[:], op=OP.add)
                        if INLINE_HEADS:
                            nc.tensor.matmul(
                                psum_hd[c][:],
                                wmsx_t[:, t * m_head:(t + 1) * m_head],
                                hnxt[:, cs],
                                start=(t == 0), stop=(t == t_steps - 1),
                                skip_group_check=True,
                            )
                    if not INLINE_HEADS:
                        # stash h_{t+1} for the post-loop head phase
                        nc.sync.dma_start(
                            hst[t],
                            hnxt[:].bitcast(F32) if PRECISE else hnxt[:])

                # ---- head psum -> SBUF ----
                if INLINE_HEADS:
                    for c in range(NF):
                        nc.vector.tensor_copy(hh[:, c * F:(c + 1) * F],
                                              psum_hd[c][:])
            if not INLINE_HEADS:
                # ---- post-loop head phase (reads h from DRAM) ----
                with tc.tile_pool(name="hrd", bufs=3) as hrd, \
                     tc.tile_pool(name="phd2", bufs=1, space="PSUM") as phd2:
                    psum_hd = [
                        phd2.tile([m_head, F], F32, name=f"pshd{c}", tag=f"pshd{c}")
                        for c in range(NF)
                    ]
                    for t in range(t_steps):
                        h_t = hrd.tile([H, BC], RD, name="hrt", tag="hrt")
                        nc.sync.dma_start(
                            h_t[:], hst[t].bitcast(RD) if PRECISE else hst[t])
                        for c in range(NF):
                            nc.tensor.matmul(
                                psum_hd[c][:],
                                wmsx_t[:, t * m_head:(t + 1) * m_head],
                                h_t[:, c * F:(c + 1) * F],
                                start=(t == 0), stop=(t == t_steps - 1),
                                skip_group_check=True,
                            )
                    for c in range(NF):
                        nc.vector.tensor_copy(hh[:, c * F:(c + 1) * F],
                                              psum_hd[c][:])

            # ---- finalize heads ----
            mu_sb = pp.tile([2 * t_steps, BC], F32)
            std_sb = pp.tile([2 * t_steps, BC], F32)
            nc.scalar.activation(mu_sb[:], hh[0:2 * t_steps, :], AF.Identity,
                                 bias=bmu_t[:])
            nc.scalar.activation(std_sb[:], hh[std_off:std_off + 2 * t_steps, :], AF.Exp,
                                 bias=bstd_t[:], scale=0.5)
            nc.sync.dma_start(omu_d, mu_sb[:])
            nc.sync.dma_start(ostd_d, std_sb[:])

    nc.compile()
    return nc


_NC_CACHE = {}


def _get_nc(debug=False):
    if "nc" not in _NC_CACHE:
        _NC_CACHE["nc"] = build_nc(debug=debug)
    return _NC_CACHE["nc"]


def make_in_maps(last_obs_state, enc_h_feat, z, sg, fut_traj,
                 W_dh, b_dh, W_vel, b_vel, W_ih, b_ih, W_hh, b_hh,
                 W_mu, b_mu, W_std, b_std, t_steps=T):
    f32 = np.float32
    bf = f32 if PRECISE else ml_dtypes.bfloat16

    # ---- weight packing (core-independent) ----
    # W_big: (KIN, 512) ; out cols = [gi_r, gi_z, gi_n, h0]
    wbig = np.zeros((KIN, 512), f32)
    wbig[0:1056, 0:384] = W_ih[:, 0:1056].T
    wbig[0:1056, 384:512] = W_dh.T
    # sg rows: rel = (sg - lo[:, :2])/dt feeds W_ih[:, 1058:1060]
    wbig[1056:1058, 0:384] = (W_ih[:, 1058:1060] / DT_CONST).T
    # lo rows (6): first two carry -W_rel/dt
    wbig[1058:1060, 0:384] = (-W_ih[:, 1058:1060] / DT_CONST).T
    # ones row: input-side biases
    wbig[1064, 0:384] = b_ih
    wbig[1064, 384:512] = b_dh

    whht = np.ascontiguousarray(W_hh.T).astype(bf)           # (128, 384)
    k3rz = np.zeros((3, 2 * H), f32)
    k3rz[0:2, 0:H] = W_ih[0:128, 1056:1058].T                # a -> r gate
    k3rz[2, 0:H] = b_hh[0:128]
    k3rz[0:2, H:2 * H] = W_ih[128:256, 1056:1058].T          # a -> z gate
    k3rz[2, H:2 * H] = b_hh[128:256]
    k2n = np.ascontiguousarray(W_ih[256:384, 1056:1058].T).astype(bf)  # (2,128)
    k3p = np.zeros((66, H), f32)
    k3p[0:3] = k3rz[:, 0:H]
    k3p[32:35] = k3rz[:, H:2 * H]
    k3p[64:66] = W_ih[256:384, 1056:1058].T
    k3p = k3p.astype(bf)
    # head lhsT variants: variant t scatters W_mu/W_std columns to output
    # rows {t, T+t, 2T+t, 3T+t}
    std_off = ((2 * t_steps + 31) // 32) * 32
    m_head = std_off + 2 * t_steps
    wmsx = np.zeros((H, t_steps, m_head), f32)
    for t in range(t_steps):
        wmsx[:, t, t] = W_mu[0]
        wmsx[:, t, t_steps + t] = W_mu[1]
        wmsx[:, t, std_off + t] = W_std[0]
        wmsx[:, t, std_off + t_steps + t] = W_std[1]
    wmsx = wmsx.reshape(H, t_steps * m_head)
    bhhn = b_hh[256:384].reshape(H, 1).astype(f32)
    bmu48 = np.repeat(b_mu, t_steps).reshape(2 * t_steps, 1).astype(f32)
    bstd48 = 0.5 * np.repeat(b_std, t_steps).reshape(2 * t_steps, 1).astype(f32)

    if PRECISE:
        ident128 = np.eye(H, dtype=f32)
    else:
        ident128 = np.eye(H).astype(ml_dtypes.bfloat16).view(np.uint16)
        ident128 = np.ascontiguousarray(
            np.concatenate([ident128, np.zeros_like(ident128)], 1).view(np.uint32)
        ).view(f32)[:, :H]  # unused path; PRECISE is the shipped config
    # host-side tiny matmul for a0 (0.4 MFLOP)
    a0 = last_obs_state @ W_vel.T + b_vel                    # (B, 2)

    in_maps = []
    for c in range(NCORES):
        sl = slice(c * BC, (c + 1) * BC)
        xt = np.empty((KIN, BC), f32)
        xt[0:MLP] = enc_h_feat[sl].T
        xt[MLP:1056] = z[sl].T
        xt[1056:1058] = sg[sl].T
        xt[1058:1064] = last_obs_state[sl].T
        xt[1064] = 1.0
        a3 = np.empty((t_steps, 3, BC), f32)
        a3[0, 0:2] = a0[sl].T
        for t in range(1, t_steps):
            a3[t, 0:2] = fut_traj[t - 1, sl, 2:4].T
        a3[:, 2] = 1.0
        in_maps.append({
            "xt": xt,
            "wbig": wbig,
            "a3": a3.astype(bf),
            "whht": whht,
            "k3rz": k3rz.astype(bf),
            "k2n": k2n,
            "k3p": k3p,
            "wmsx": wmsx.astype(bf),
            "ident128": ident128,
            "bhhn": bhhn,
            "bmu48": bmu48,
            "bstd48": bstd48,
        })
    return in_maps


def unpack_outputs(results, t_steps=T):
    mus = np.empty((t_steps, B, 2), np.float32)
    stds = np.empty((t_steps, B, 2), np.float32)
    for c in range(NCORES):
        sl = slice(c * BC, (c + 1) * BC)
        omu = results[c]["omu"].reshape(2, t_steps, BC)
        ostd = results[c]["ostd"].reshape(2, t_steps, BC)
        mus[:, sl, 0] = omu[0]
        mus[:, sl, 1] = omu[1]
        stds[:, sl, 0] = ostd[0]
        stds[:, sl, 1] = ostd[1]
    return mus, stds


def kernel(last_obs_state, enc_h_feat, z, sg, fut_traj,
           W_dh, b_dh, W_vel, b_vel, W_ih, b_ih, W_hh, b_hh,
           W_mu, b_mu, W_std, b_std):
    args = dict(
        last_obs_state=np.asarray(last_obs_state, np.float32),
        enc_h_feat=np.asarray(enc_h_feat, np.float32),
        z=np.asarray(z, np.float32),
        sg=np.asarray(sg, np.float32),
        fut_traj=np.asarray(fut_traj, np.float32),
        W_dh=np.asarray(W_dh, np.float32), b_dh=np.asarray(b_dh, np.float32),
        W_vel=np.asarray(W_vel, np.float32), b_vel=np.asarray(b_vel, np.float32),
        W_ih=np.asarray(W_ih, np.float32), b_ih=np.asarray(b_ih, np.float32),
        W_hh=np.asarray(W_hh, np.float32), b_hh=np.asarray(b_hh, np.float32),
        W_mu=np.asarray(W_mu, np.float32), b_mu=np.asarray(b_mu, np.float32),
        W_std=np.asarray(W_std, np.float32), b_std=np.asarray(b_std, np.float32),
    )
    nc = _get_nc()
    in_maps = make_in_maps(**args)
    res = run_bass_kernel_spmd(nc, in_maps, core_ids=list(range(NCORES)))
    return unpack_outputs(res.results)



# revision 5
# speedup vs baseline: 1.3874x; 1.3874x over previous
"""Trainium2 Bass/Tile kernel for nn_Decoder (GRU decoder with teacher forcing).

Math (per reference):
  zx  = [enc_h_feat, z]                    (B, 1056)
  h0  = zx @ W_dh.T + b_dh                 (B, 128)
  a0  = last_obs @ W_vel.T + b_vel         (B, 2)
  rel = (sg - last_obs[:, :2]) / dt        (B, 2)
  a_t = a0 if t==0 else fut_traj[t-1,:,2:4]
  x_t = [zx, a_t, rel]  -> GRUCell(x_t, h) -> mu_t, std_t

Device strategy (8 cores, batch-sharded, 2048 rows/core), all fp16
operands with fp32 PSUM accumulation:
  - Feature-on-partition, batch-on-free layout; free chunks of 512.
  - Setup: [gi_r|gi_z|gi_n|h0](512 rows) = W_big.T @ XT with K=1065
    host-packed rows [zxT; sgT; loT; ones].  The rel term and all
    input-side biases are folded into W_big on the host.
  - Per step/chunk i=(t,c), software-pipelined one chunk deep so PE's
    in-order stream never waits on the sigmoid->q chain:
      stage A(i):  psum_rz  = Whh_{r,z}@h + K3@[a;1] + I@gi_{r,z}   [PE x6]
                   psum_hn  = Whh_n@h                               [PE x1]
                   rz       = sigmoid(psum_rz)                      [ScalarE]
                   psum_gin = (psum_hn + b_hh_n) * r                [DVE stt,
                              written straight into PSUM: warm-start]
      stage B(i-1): psum_gin += I@gi_n + K2@a                       [PE x2]
                   n  = tanh(psum_gin)                              [ScalarE]
                   d = h - n, e = z*d, h' = n + e    [DVE 2-byte 2x tt ops]
    h' for every step is kept in SBUF (25 x 0.5MB fp16 tiles), so no
    PSUM bank is pinned during the loop: rz/hn/gin pools all run bufs=2
    and PE never stalls on a WAR against the activation reads.
  - Post-loop: per-step head matmuls with a scattered-column lhsT
    accumulate mu/std pre-activations for ALL steps into 4 PSUM tiles;
    mu = Identity(+b_mu), std = Exp(0.5*(.)+0.5*b_std) = sqrt(exp(.)).
  - GPSIMD does no work (its tensor ops are ~3-5x slower than DVE);
    DMAs issue from SP/Activation sequencers (HWDGE path, no engine cost).
Host does only sharding/transposes/weight packing (a0 is a (B,6)@(6,2)
matmul on host, ~0.4 MFLOP, negligible vs the 52 GFLOP kernel).
"""

import numpy as np

import concourse.bass as bass
import concourse.mybir as mybir
import concourse.tile as tile
from concourse import bacc
from concourse.bass_utils import run_bass_kernel_spmd

F32 = mybir.dt.float32
F16 = mybir.dt.float16
AF = mybir.ActivationFunctionType
OP = mybir.AluOpType

B, T, MLP, ZD, H, NS, NP = 16384, 24, 1024, 32, 128, 6, 2
NCORES = 8
BC = B // NCORES            # 2048 rows per core
F = 512                     # free-dim chunk
NF = BC // F                # 4 chunks
KIN = MLP + ZD + NP + NS + 1  # 1065 = zx(1056) + sg(2) + lo(6) + ones(1)
NKC = (KIN + 127) // 128    # 9 K-chunks (8x128 + 41)
DT_CONST = 0.4 * 12

# q -> PSUM warm-start (DVE writes psum_gin, PE accumulates on top).
# False = q goes to SBUF and is injected with an ident matmul.
WARMSTART = False


def build_nc(debug=False, t_steps=T):
    HD = F16
    nc = bacc.Bacc("TRN2", target_bir_lowering=False, debug=debug)

    # ---- DRAM I/O ----
    xt_d = nc.dram_tensor("xt", [KIN, BC], F16, kind="ExternalInput").ap()
    wbig_d = nc.dram_tensor("wbig", [KIN, 512], F16, kind="ExternalInput").ap()
    a3_d = nc.dram_tensor("a3", [t_steps, 3, BC], F16, kind="ExternalInput").ap()
    whht_d = nc.dram_tensor("whht", [H, 3 * H], F16, kind="ExternalInput").ap()
    k3p_d = nc.dram_tensor("k3p", [66, H], F16, kind="ExternalInput").ap()
    _std_off = ((2 * t_steps + 31) // 32) * 32
    _m_head = _std_off + 2 * t_steps
    wmsx_d = nc.dram_tensor("wmsx", [H, t_steps * _m_head], F16,
                            kind="ExternalInput").ap()
    id_d = nc.dram_tensor("identh", [H, H], F16, kind="ExternalInput").ap()
    bhhn_d = nc.dram_tensor("bhhn", [H, 1], F32, kind="ExternalInput").ap()
    bmu_d = nc.dram_tensor("bmu48", [2 * t_steps, 1], F32, kind="ExternalInput").ap()
    bstd_d = nc.dram_tensor("bstd48", [2 * t_steps, 1], F32, kind="ExternalInput").ap()
    omu_d = nc.dram_tensor("omu", [2 * t_steps, BC], F32, kind="ExternalOutput").ap()
    ostd_d = nc.dram_tensor("ostd", [2 * t_steps, BC], F32, kind="ExternalOutput").ap()

    with tile.TileContext(nc) as tc:
        with tc.tile_pool(name="persist", bufs=1) as pp:
            # persistent SBUF state
            gi_r = pp.tile([H, BC], HD)
            gi_z = pp.tile([H, BC], HD)
            gi_n = pp.tile([H, BC], HD)
            # h history: one tile per step boundary (h_state[t] = h before
            # step t); separate tiles keep dependency tracking per-step
            hst = [pp.tile([H, BC], HD, name=f"hst{t}") for t in range(t_steps + 1)]
            std_off = ((2 * t_steps + 31) // 32) * 32
            m_head = std_off + 2 * t_steps
            hh = pp.tile([m_head, BC], F32, name="headacc")
            whht_t = pp.tile([H, 3 * H], HD)
            k3p_t = pp.tile([66, H], HD)
            wmsx_t = pp.tile([H, t_steps * m_head], HD)
            bhhn_t = pp.tile([H, 1], F32)
            bmu_t = pp.tile([2 * t_steps, 1], F32)
            bstd_t = pp.tile([2 * t_steps, 1], F32)
            ident = pp.tile([H, H], HD)

            nc.sync.dma_start(whht_t[:], whht_d)
            nc.sync.dma_start(k3p_t[:], k3p_d)
            nc.scalar.dma_start(wmsx_t[:], wmsx_d)
            nc.sync.dma_start(bhhn_t[:], bhhn_d)
            nc.sync.dma_start(bmu_t[:], bmu_d)
            nc.sync.dma_start(bstd_t[:], bstd_d)
            nc.sync.dma_start(ident[:], id_d)

            gi_dst = [gi_r, gi_z, gi_n, None]

            # ---- setup: [gi | h0] = W_big.T @ XT  (fp16) ----
            with tc.tile_pool(name="xtp", bufs=1) as xtp, \
                 tc.tile_pool(name="wp", bufs=1) as wp, \
                 tc.tile_pool(name="sps", bufs=4, space="PSUM") as sps:
                # one wide DMA per K-chunk (desc size 4KB): DMA-count, not
                # bytes, dominated the old per-(k,chunk) loads
                xt_tiles = []
                w_tiles = []
                for k in range(NKC):
                    kc = min(128, KIN - 128 * k)
                    xt_k = xtp.tile([kc, BC], HD, name=f"xt{k}", tag=f"xt{k}")
                    _eng = [nc.scalar, nc.sync][k % 2]
                    _eng.dma_start(xt_k[:], xt_d[128 * k:128 * k + kc, :])
                    xt_tiles.append(xt_k)
                    w_k = wp.tile([kc, 512], HD, name=f"w{k}", tag=f"w{k}")
                    _eng = [nc.sync, nc.scalar][k % 2]
                    _eng.dma_start(w_k[:], wbig_d[128 * k:128 * k + kc, :])
                    w_tiles.append(w_k)

                for m in range(4):
                    for nci in range(NF):
                        ps = sps.tile([128, F], F32, name="setps", tag="setps")
                        for k in range(NKC):
                            nc.tensor.matmul(
                                ps[:], w_tiles[k][:, 128 * m:128 * (m + 1)],
                                xt_tiles[k][:, nci * F:(nci + 1) * F],
                                start=(k == 0), stop=(k == NKC - 1),
                            )
                        dst = gi_dst[m] if m < 3 else hst[0]
                        nc.vector.tensor_copy(dst[:, nci * F:(nci + 1) * F], ps[:])

            # ---- recurrence, software-pipelined one chunk deep ----
            NI = t_steps * NF
            with tc.tile_pool(name="a3p", bufs=3) as a3p, \
                 tc.tile_pool(name="gp", bufs=3) as gp, \
                 tc.tile_pool(name="prz", bufs=2, space="PSUM") as prz, \
                 tc.tile_pool(name="phn", bufs=2, space="PSUM") as phn, \
                 tc.tile_pool(name="pgin", bufs=2, space="PSUM") as pgin:
                a3_tiles = {}
                live = {}   # flat index -> dict of tiles for stage B
                for i in range(NI + 1):
                    # ---- stage A for flat index i ----
                    if i < NI:
                        t, c = divmod(i, NF)
                        cs = slice(c * F, (c + 1) * F)
                        hs = hst[t][:, cs]
                        if c == 0:
                            a3_t = a3p.tile([66, BC], HD, name="a3t", tag="a3t")
                            nc.sync.dma_start(a3_t[0:3, :], a3_d[t])
                            nc.sync.dma_start(a3_t[32:35, :], a3_d[t])
                            nc.sync.dma_start(a3_t[64:66, :], a3_d[t][0:2])
                            a3_tiles[t] = a3_t
                        a3_t = a3_tiles[t]
                        psum_rz = prz.tile([128, 2 * F], F32, name="psrz", tag="psrz")
                        nc.tensor.matmul(psum_rz[:, 0:F], whht_t[:, 0:H], hs,
                                         start=True, stop=False)
                        nc.tensor.matmul(psum_rz[:, F:2 * F], whht_t[:, H:2 * H], hs,
                                         start=True, stop=False)
                        nc.tensor.matmul(psum_rz[:, 0:F], k3p_t[0:3, :],
                                         a3_t[0:3, cs], start=False, stop=False)
                        nc.tensor.matmul(psum_rz[:, F:2 * F], k3p_t[32:35, :],
                                         a3_t[32:35, cs], start=False, stop=False)
                        nc.tensor.matmul(psum_rz[:, 0:F], ident[:], gi_r[:, cs],
                                         start=False, stop=True)
                        nc.tensor.matmul(psum_rz[:, F:2 * F], ident[:], gi_z[:, cs],
                                         start=False, stop=True)
                        psum_hn = phn.tile([128, F], F32, name="pshn", tag="pshn")
                        nc.tensor.matmul(psum_hn[:], whht_t[:, 2 * H:3 * H], hs,
                                         start=True, stop=True)
                        rz = gp.tile([128, 2 * F], HD, name="rz", tag="rz")
                        nc.scalar.activation(rz[:], psum_rz[:], AF.Sigmoid)
                        # q = (gh_n + b_hh_n) * r written straight into the
                        # gin PSUM bank: stage B matmuls accumulate on top
                        psum_gin = pgin.tile([128, F], F32, name="psgin", tag="psgin")
                        if WARMSTART:
                            nc.vector.scalar_tensor_tensor(
                                psum_gin[:], psum_hn[:], bhhn_t[:], rz[:, 0:F],
                                op0=OP.add, op1=OP.mult,
                            )
                            q = None
                        else:
                            q = gp.tile([128, F], HD, name="q", tag="q")
                            nc.vector.scalar_tensor_tensor(
                                q[:], psum_hn[:], bhhn_t[:], rz[:, 0:F],
                                op0=OP.add, op1=OP.mult,
                            )
                        live[i] = dict(t=t, cs=cs, hs=hs, rz=rz, a3=a3_t,
                                       pgin=psum_gin, q=q)
                    # ---- stage B for flat index i-1 ----
                    if i >= 1:
                        lv = live.pop(i - 1)
                        t, cs = lv["t"], lv["cs"]
                        psum_gin = lv["pgin"]
                        nc.tensor.matmul(psum_gin[:], ident[:], gi_n[:, cs],
                                         start=not WARMSTART, stop=False,
                                         skip_group_check=WARMSTART)
                        nc.tensor.matmul(psum_gin[:], k3p_t[64:66, :],
                                         lv["a3"][64:66, cs], start=False,
                                         stop=WARMSTART,
                                         skip_group_check=WARMSTART)
                        if not WARMSTART:
                            nc.tensor.matmul(psum_gin[:], ident[:], lv["q"][:],
                                             start=False, stop=True)
                        nt = gp.tile([128, F], HD, name="nt", tag="nt")
                        nc.scalar.activation(nt[:], psum_gin[:], AF.Tanh)
                        d = gp.tile([128, F], HD, name="d", tag="d")
                        nc.vector.tensor_tensor(d[:], lv["hs"], nt[:], op=OP.subtract)
                        e = gp.tile([128, F], HD, name="e", tag="e")
                        nc.vector.tensor_tensor(e[:], lv["rz"][:, F:2 * F], d[:],
                                                op=OP.mult)
                        nc.vector.tensor_tensor(hst[t + 1][:, cs], nt[:], e[:],
                                                op=OP.add)

            # ---- post-loop heads: accumulate over all steps in PSUM ----
            with tc.tile_pool(name="phd", bufs=1, space="PSUM") as phd:
                psum_hd = [
                    phd.tile([m_head, F], F32, name=f"pshd{c}", tag=f"pshd{c}")
                    for c in range(NF)
                ]
                for t in range(t_steps):
                    for c in range(NF):
                        nc.tensor.matmul(
                            psum_hd[c][:],
                            wmsx_t[:, t * m_head:(t + 1) * m_head],
                            hst[t + 1][:, c * F:(c + 1) * F],
                            start=(t == 0), stop=(t == t_steps - 1),
                            skip_group_check=True,
                        )
                for c in range(NF):
                    nc.vector.tensor_copy(hh[:, c * F:(c + 1) * F],
                                          psum_hd[c][:])

            # ---- finalize heads ----
            mu_sb = pp.tile([2 * t_steps, BC], F32)
            std_sb = pp.tile([2 * t_steps, BC], F32)
            nc.scalar.activation(mu_sb[:], hh[0:2 * t_steps, :], AF.Identity,
                                 bias=bmu_t[:])
            nc.scalar.activation(std_sb[:], hh[std_off:std_off + 2 * t_steps, :], AF.Exp,
                                 bias=bstd_t[:], scale=0.5)
            nc.sync.dma_start(omu_d, mu_sb[:])
            nc.sync.dma_start(ostd_d, std_sb[:])

    nc.compile()
    return nc


_NC_CACHE = {}


def _get_nc(debug=False):
    if "nc" not in _NC_CACHE:
        _NC_CACHE["nc"] = build_nc(debug=debug)
    return _NC_CACHE["nc"]


def make_in_maps(last_obs_state, enc_h_feat, z, sg, fut_traj,
                 W_dh, b_dh, W_vel, b_vel, W_ih, b_ih, W_hh, b_hh,
                 W_mu, b_mu, W_std, b_std, t_steps=T):
    f32 = np.float32
    f16 = np.float16

    # ---- weight packing (core-independent) ----
    # W_big: (KIN, 512) ; out cols = [gi_r, gi_z, gi_n, h0]
    wbig = np.zeros((KIN, 512), f32)
    wbig[0:1056, 0:384] = W_ih[:, 0:1056].T
    wbig[0:1056, 384:512] = W_dh.T
    # sg rows: rel = (sg - lo[:, :2])/dt feeds W_ih[:, 1058:1060]
    wbig[1056:1058, 0:384] = (W_ih[:, 1058:1060] / DT_CONST).T
    # lo rows (6): first two carry -W_rel/dt
    wbig[1058:1060, 0:384] = (-W_ih[:, 1058:1060] / DT_CONST).T
    # ones row: input-side biases
    wbig[1064, 0:384] = b_ih
    wbig[1064, 384:512] = b_dh
    wbig = wbig.astype(f16)

    whht = np.ascontiguousarray(W_hh.T).astype(f16)          # (128, 384)
    k3p = np.zeros((66, H), f32)
    k3p[0:2, 0:H] = W_ih[0:128, 1056:1058].T                 # a -> r gate
    k3p[2, 0:H] = b_hh[0:128]
    k3p[32:34] = W_ih[128:256, 1056:1058].T                  # a -> z gate
    k3p[34] = b_hh[128:256]
    k3p[64:66] = W_ih[256:384, 1056:1058].T                  # a -> n gate
    k3p = k3p.astype(f16)
    # head lhsT variants: variant t scatters W_mu/W_std columns to output
    # rows {t, T+t, 2T+t, 3T+t}
    std_off = ((2 * t_steps + 31) // 32) * 32
    m_head = std_off + 2 * t_steps
    wmsx = np.zeros((H, t_steps, m_head), f32)
    for t in range(t_steps):
        wmsx[:, t, t] = W_mu[0]
        wmsx[:, t, t_steps + t] = W_mu[1]
        wmsx[:, t, std_off + t] = W_std[0]
        wmsx[:, t, std_off + t_steps + t] = W_std[1]
    wmsx = wmsx.reshape(H, t_steps * m_head).astype(f16)
    bhhn = b_hh[256:384].reshape(H, 1).astype(f32)
    bmu48 = np.repeat(b_mu, t_steps).reshape(2 * t_steps, 1).astype(f32)
    bstd48 = 0.5 * np.repeat(b_std, t_steps).reshape(2 * t_steps, 1).astype(f32)

    identh = np.eye(H, dtype=f16)
    # host-side tiny matmul for a0 (0.4 MFLOP)
    a0 = last_obs_state @ W_vel.T + b_vel                    # (B, 2)

    in_maps = []
    for c in range(NCORES):
        sl = slice(c * BC, (c + 1) * BC)
        xt = np.empty((KIN, BC), f32)
        xt[0:MLP] = enc_h_feat[sl].T
        xt[MLP:1056] = z[sl].T
        xt[1056:1058] = sg[sl].T
        xt[1058:1064] = last_obs_state[sl].T
        xt[1064] = 1.0
        a3 = np.empty((t_steps, 3, BC), f32)
        a3[0, 0:2] = a0[sl].T
        for t in range(1, t_steps):
            a3[t, 0:2] = fut_traj[t - 1, sl, 2:4].T
        a3[:, 2] = 1.0
        in_maps.append({
            "xt": xt.astype(f16),
            "wbig": wbig,
            "a3": a3.astype(f16),
            "whht": whht,
            "k3p": k3p,
            "wmsx": wmsx,
            "identh": identh,
            "bhhn": bhhn,
            "bmu48": bmu48,
            "bstd48": bstd48,
        })
    return in_maps


def unpack_outputs(results, t_steps=T):
    mus = np.empty((t_steps, B, 2), np.float32)
    stds = np.empty((t_steps, B, 2), np.float32)
    for c in range(NCORES):
        sl = slice(c * BC, (c + 1) * BC)
        omu = results[c]["omu"].reshape(2, t_steps, BC)
        ostd = results[c]["ostd"].reshape(2, t_steps, BC)
        mus[:, sl, 0] = omu[0]
        mus[:, sl, 1] = omu[1]
        stds[:, sl, 0] = ostd[0]
        stds[:, sl, 1] = ostd[1]
    return mus, stds


def kernel(last_obs_state, enc_h_feat, z, sg, fut_traj,
           W_dh, b_dh, W_vel, b_vel, W_ih, b_ih, W_hh, b_hh,
           W_mu, b_mu, W_std, b_std):
    args = dict(
        last_obs_state=np.asarray(last_obs_state, np.float32),
        enc_h_feat=np.asarray(enc_h_feat, np.float32),
        z=np.asarray(z, np.float32),
        sg=np.asarray(sg, np.float32),
        fut_traj=np.asarray(fut_traj, np.float32),
        W_dh=np.asarray(W_dh, np.float32), b_dh=np.asarray(b_dh, np.float32),
        W_vel=np.asarray(W_vel, np.float32), b_vel=np.asarray(b_vel, np.float32),
        W_ih=np.asarray(W_ih, np.float32), b_ih=np.asarray(b_ih, np.float32),
        W_hh=np.asarray(W_hh, np.float32), b_hh=np.asarray(b_hh, np.float32),
        W_mu=np.asarray(W_mu, np.float32), b_mu=np.asarray(b_mu, np.float32),
        W_std=np.asarray(W_std, np.float32), b_std=np.asarray(b_std, np.float32),
    )
    nc = _get_nc()
    in_maps = make_in_maps(**args)
    res = run_bass_kernel_spmd(nc, in_maps, core_ids=list(range(NCORES)))
    return unpack_outputs(res.results)


# revision 6
# speedup vs baseline: 1.4963x; 1.0785x over previous
"""Trainium2 Bass/Tile kernel for nn_Decoder (GRU decoder with teacher forcing).

Math (per reference):
  zx  = [enc_h_feat, z]                    (B, 1056)
  h0  = zx @ W_dh.T + b_dh                 (B, 128)
  a0  = last_obs @ W_vel.T + b_vel         (B, 2)
  rel = (sg - last_obs[:, :2]) / dt        (B, 2)
  a_t = a0 if t==0 else fut_traj[t-1,:,2:4]
  x_t = [zx, a_t, rel]  -> GRUCell(x_t, h) -> mu_t, std_t

Device strategy (8 cores, batch-sharded, 2048 rows/core), all fp16
operands with fp32 PSUM accumulation:
  - Feature-on-partition, batch-on-free layout; free chunks of 512.
  - Setup: [gi_r|gi_z|gi_n|h0](512 rows) = W_big.T @ XT with K=1065
    host-packed rows [zxT; sgT; loT; ones].  The rel term and all
    input-side biases are folded into W_big on the host.
  - Per step/chunk i=(t,c), software-pipelined one chunk deep so PE's
    in-order stream never waits on the sigmoid->q chain:
      stage A(i):  psum_rz  = Whh_{r,z}@h + K3@[a;1] + I@gi_{r,z}   [PE x6]
                   psum_hn  = Whh_n@h                               [PE x1]
                   rz       = sigmoid(psum_rz)                      [ScalarE]
                   psum_gin = (psum_hn + b_hh_n) * r                [DVE stt,
                              written straight into PSUM: warm-start]
      stage B(i-1): psum_gin += I@gi_n + K2@a                       [PE x2]
                   n  = tanh(psum_gin)                              [ScalarE]
                   d = h - n, e = z*d, h' = n + e    [DVE 2-byte 2x tt ops]
    h' for every step is kept in SBUF (25 x 0.5MB fp16 tiles), so no
    PSUM bank is pinned during the loop: rz/hn/gin pools all run bufs=2
    and PE never stalls on a WAR against the activation reads.
  - Post-loop: per-step head matmuls with a scattered-column lhsT
    accumulate mu/std pre-activations for ALL steps into 4 PSUM tiles;
    mu = Identity(+b_mu), std = Exp(0.5*(.)+0.5*b_std) = sqrt(exp(.)).
  - GPSIMD does no work (its tensor ops are ~3-5x slower than DVE);
    DMAs issue from SP/Activation sequencers (HWDGE path, no engine cost).
Host does only sharding/transposes/weight packing (a0 is a (B,6)@(6,2)
matmul on host, ~0.4 MFLOP, negligible vs the 52 GFLOP kernel).
"""

import numpy as np

import concourse.bass as bass
import concourse.mybir as mybir
import concourse.tile as tile
from concourse import bacc
from concourse.bass_utils import run_bass_kernel_spmd

F32 = mybir.dt.float32
F16 = mybir.dt.float16
AF = mybir.ActivationFunctionType
OP = mybir.AluOpType

B, T, MLP, ZD, H, NS, NP = 16384, 24, 1024, 32, 128, 6, 2
NCORES = 8
BC = B // NCORES            # 2048 rows per core
F = 512                     # free-dim chunk
NF = BC // F                # 4 chunks
KIN = MLP + ZD + NP + NS + 1  # 1065 = zx(1056) + sg(2) + lo(6) + ones(1)
NKC = (KIN + 127) // 128    # 9 K-chunks (8x128 + 41)
DT_CONST = 0.4 * 12


def build_nc(debug=False, t_steps=T):
    HD = F16
    nc = bacc.Bacc("TRN2", target_bir_lowering=False, debug=debug)

    # ---- DRAM I/O ----
    xt_d = nc.dram_tensor("xt", [KIN, BC], F16, kind="ExternalInput").ap()
    wbig_d = nc.dram_tensor("wbig", [KIN, 512], F16, kind="ExternalInput").ap()
    a3_d = nc.dram_tensor("a3", [t_steps, 3, BC], F16, kind="ExternalInput").ap()
    whht_d = nc.dram_tensor("whht", [H, 3 * H], F16, kind="ExternalInput").ap()
    k3p_d = nc.dram_tensor("k3p", [66, H], F16, kind="ExternalInput").ap()
    _std_off = ((2 * t_steps + 31) // 32) * 32
    _m_head = _std_off + 2 * t_steps
    wmsx_d = nc.dram_tensor("wmsx", [H, t_steps * _m_head], F16,
                            kind="ExternalInput").ap()
    id_d = nc.dram_tensor("identh", [H, H], F16, kind="ExternalInput").ap()
    bhhn_d = nc.dram_tensor("bhhn", [H, 1], F32, kind="ExternalInput").ap()
    bmu_d = nc.dram_tensor("bmu48", [2 * t_steps, 1], F32, kind="ExternalInput").ap()
    bstd_d = nc.dram_tensor("bstd48", [2 * t_steps, 1], F32, kind="ExternalInput").ap()
    omu_d = nc.dram_tensor("omu", [2 * t_steps, BC], F32, kind="ExternalOutput").ap()
    ostd_d = nc.dram_tensor("ostd", [2 * t_steps, BC], F32, kind="ExternalOutput").ap()

    with tile.TileContext(nc) as tc:
        with tc.tile_pool(name="persist", bufs=1) as pp:
            # persistent SBUF state
            gi_r = pp.tile([H, BC], HD)
            gi_z = pp.tile([H, BC], HD)
            gi_n = pp.tile([H, BC], HD)
            # h history: one tile per step boundary (h_state[t] = h before
            # step t); separate tiles keep dependency tracking per-step
            hst = [pp.tile([H, BC], HD, name=f"hst{t}") for t in range(t_steps + 1)]
            std_off = ((2 * t_steps + 31) // 32) * 32
            m_head = std_off + 2 * t_steps
            whht_t = pp.tile([H, 3 * H], HD)
            k3p_t = pp.tile([66, H], HD)
            wmsx_t = pp.tile([H, t_steps * m_head], HD)
            bhhn_t = pp.tile([H, 1], F32)
            bmu_t = pp.tile([2 * t_steps, 1], F32)
            bstd_t = pp.tile([2 * t_steps, 1], F32)
            ident = pp.tile([H, H], HD)

            gi_dst = [gi_r, gi_z, gi_n, None]

            # ---- setup: [gi | h0] = W_big.T @ XT  (fp16) ----
            with tc.tile_pool(name="xtp", bufs=1) as xtp, \
                 tc.tile_pool(name="wp", bufs=1) as wp, \
                 tc.tile_pool(name="sps", bufs=4, space="PSUM") as sps:
                # one wide DMA per K-chunk (desc size 4KB): DMA-count, not
                # bytes, dominated the old per-(k,chunk) loads
                xt_tiles = []
                w_tiles = []
                for k in range(NKC):
                    kc = min(128, KIN - 128 * k)
                    xt_k = xtp.tile([kc, BC], HD, name=f"xt{k}", tag=f"xt{k}")
                    _eng = [nc.scalar, nc.sync][k % 2]
                    _eng.dma_start(xt_k[:], xt_d[128 * k:128 * k + kc, :])
                    xt_tiles.append(xt_k)
                    w_k = wp.tile([kc, 512], HD, name=f"w{k}", tag=f"w{k}")
                    _eng = [nc.sync, nc.scalar][k % 2]
                    _eng.dma_start(w_k[:], wbig_d[128 * k:128 * k + kc, :])
                    w_tiles.append(w_k)
                # loop/head weights: issued after the setup operands so the
                # first setup matmul starts as early as possible
                nc.sync.dma_start(ident[:], id_d)
                nc.sync.dma_start(whht_t[:], whht_d)
                nc.sync.dma_start(k3p_t[:], k3p_d)
                nc.sync.dma_start(bhhn_t[:], bhhn_d)
                nc.scalar.dma_start(wmsx_t[:], wmsx_d)
                nc.scalar.dma_start(bmu_t[:], bmu_d)
                nc.scalar.dma_start(bstd_t[:], bstd_d)

                for m in range(4):
                    for nci in range(NF):
                        ps = sps.tile([128, F], F32, name="setps", tag="setps")
                        for k in range(NKC):
                            nc.tensor.matmul(
                                ps[:], w_tiles[k][:, 128 * m:128 * (m + 1)],
                                xt_tiles[k][:, nci * F:(nci + 1) * F],
                                start=(k == 0), stop=(k == NKC - 1),
                            )
                        dst = gi_dst[m] if m < 3 else hst[0]
                        nc.vector.tensor_copy(dst[:, nci * F:(nci + 1) * F], ps[:])

            # ---- recurrence, software-pipelined one chunk deep ----
            NI = t_steps * NF
            with tc.tile_pool(name="a3p", bufs=3) as a3p, \
                 tc.tile_pool(name="gp", bufs=3) as gp, \
                 tc.tile_pool(name="prz", bufs=2, space="PSUM") as prz, \
                 tc.tile_pool(name="phn", bufs=2, space="PSUM") as phn, \
                 tc.tile_pool(name="pgin", bufs=2, space="PSUM") as pgin:
                a3_tiles = {}
                live = {}   # flat index -> dict of tiles for stage B
                for i in range(NI + 2):
                    # ---- stage A for flat index i ----
                    if i < NI:
                        t, c = divmod(i, NF)
                        cs = slice(c * F, (c + 1) * F)
                        hs = hst[t][:, cs]
                        if c == 0:
                            a3_t = a3p.tile([66, BC], HD, name="a3t", tag="a3t")
                            nc.sync.dma_start(a3_t[0:3, :], a3_d[t])
                            nc.sync.dma_start(a3_t[32:35, :], a3_d[t])
                            nc.sync.dma_start(a3_t[64:66, :], a3_d[t][0:2])
                            a3_tiles[t] = a3_t
                        a3_t = a3_tiles[t]
                        psum_rz = prz.tile([128, 2 * F], F32, name="psrz", tag="psrz")
                        nc.tensor.matmul(psum_rz[:, 0:F], whht_t[:, 0:H], hs,
                                         start=True, stop=False)
                        nc.tensor.matmul(psum_rz[:, F:2 * F], whht_t[:, H:2 * H], hs,
                                         start=True, stop=False)
                        nc.tensor.matmul(psum_rz[:, 0:F], k3p_t[0:3, :],
                                         a3_t[0:3, cs], start=False, stop=False)
                        nc.tensor.matmul(psum_rz[:, F:2 * F], k3p_t[32:35, :],
                                         a3_t[32:35, cs], start=False, stop=False)
                        nc.tensor.matmul(psum_rz[:, 0:F], ident[:], gi_r[:, cs],
                                         start=False, stop=True)
                        nc.tensor.matmul(psum_rz[:, F:2 * F], ident[:], gi_z[:, cs],
                                         start=False, stop=True)
                        psum_hn = phn.tile([128, F], F32, name="pshn", tag="pshn")
                        nc.tensor.matmul(psum_hn[:], whht_t[:, 2 * H:3 * H], hs,
                                         start=True, stop=True)
                        rz = gp.tile([128, 2 * F], HD, name="rz", tag="rz")
                        nc.scalar.activation(rz[:], psum_rz[:], AF.Sigmoid)
                        q = gp.tile([128, F], HD, name="q", tag="q")
                        nc.vector.scalar_tensor_tensor(
                            q[:], psum_hn[:], bhhn_t[:], rz[:, 0:F],
                            op0=OP.add, op1=OP.mult,
                        )
                        live[i] = dict(t=t, cs=cs, hs=hs, rz=rz, a3=a3_t, q=q)
                    # ---- stage B for flat index i-1 ----
                    if 1 <= i <= NI:
                        lv = live[i - 1]
                        t, cs = lv["t"], lv["cs"]
                        # q2 = q + gi_n on DVE replaces a third PE inject
                        q2 = gp.tile([128, F], HD, name="q2", tag="q2")
                        nc.vector.tensor_tensor(q2[:], lv["q"][:], gi_n[:, cs],
                                                op=OP.add)
                        psum_gin = pgin.tile([128, F], F32, name="psgin",
                                             tag="psgin")
                        nc.tensor.matmul(psum_gin[:], k3p_t[64:66, :],
                                         lv["a3"][64:66, cs], start=True,
                                         stop=False)
                        nc.tensor.matmul(psum_gin[:], ident[:], q2[:],
                                         start=False, stop=True)
                        nt = gp.tile([128, F], HD, name="nt", tag="nt")
                        nc.scalar.activation(nt[:], psum_gin[:], AF.Tanh)
                        d = gp.tile([128, F], HD, name="d", tag="d")
                        nc.vector.tensor_tensor(d[:], lv["hs"], nt[:], op=OP.subtract)
                        # e on the otherwise-idle GPSIMD; h' lands one slot
                        # later so the Pool->DVE hop never stalls DVE's queue
                        e = gp.tile([128, F], HD, name="e", tag="e")
                        nc.gpsimd.tensor_tensor(e[:], lv["rz"][:, F:2 * F], d[:],
                                                op=OP.mult)
                        lv["nt"] = nt
                        lv["e"] = e
                    # ---- stage C for flat index i-2 ----
                    if i >= 2:
                        lv = live.pop(i - 2)
                        nc.vector.tensor_tensor(hst[lv["t"] + 1][:, lv["cs"]],
                                                lv["nt"][:], lv["e"][:], op=OP.add)

            # ---- post-loop heads: c-outer so each chunk's finalize
            # (ACT straight from PSUM + out-DMA) overlaps the next chunk's
            # 24-matmul accumulation stream ----
            with tc.tile_pool(name="phd", bufs=2, space="PSUM") as phd, \
                 tc.tile_pool(name="fin", bufs=2) as fin:
                for c in range(NF):
                    cs = slice(c * F, (c + 1) * F)
                    ps = phd.tile([m_head, F], F32, name="pshd", tag="pshd")
                    for t in range(t_steps):
                        nc.tensor.matmul(
                            ps[:],
                            wmsx_t[:, t * m_head:(t + 1) * m_head],
                            hst[t + 1][:, cs],
                            start=(t == 0), stop=(t == t_steps - 1),
                        )
                    mu_c = fin.tile([2 * t_steps, F], F32, name="muc", tag="muc")
                    std_c = fin.tile([2 * t_steps, F], F32, name="stdc", tag="stdc")
                    nc.scalar.activation(mu_c[:], ps[0:2 * t_steps, :],
                                         AF.Identity, bias=bmu_t[:])
                    nc.scalar.activation(std_c[:],
                                         ps[std_off:std_off + 2 * t_steps, :],
                                         AF.Exp, bias=bstd_t[:], scale=0.5)
                    nc.sync.dma_start(omu_d[:, cs], mu_c[:])
                    nc.sync.dma_start(ostd_d[:, cs], std_c[:])

    nc.compile()
    return nc


_NC_CACHE = {}


def _get_nc(debug=False):
    if "nc" not in _NC_CACHE:
        _NC_CACHE["nc"] = build_nc(debug=debug)
    return _NC_CACHE["nc"]


def make_in_maps(last_obs_state, enc_h_feat, z, sg, fut_traj,
                 W_dh, b_dh, W_vel, b_vel, W_ih, b_ih, W_hh, b_hh,
                 W_mu, b_mu, W_std, b_std, t_steps=T):
    f32 = np.float32
    f16 = np.float16

    # ---- weight packing (core-independent) ----
    # W_big: (KIN, 512) ; out cols = [gi_r, gi_z, gi_n, h0]
    wbig = np.zeros((KIN, 512), f32)
    wbig[0:1056, 0:384] = W_ih[:, 0:1056].T
    wbig[0:1056, 384:512] = W_dh.T
    # sg rows: rel = (sg - lo[:, :2])/dt feeds W_ih[:, 1058:1060]
    wbig[1056:1058, 0:384] = (W_ih[:, 1058:1060] / DT_CONST).T
    # lo rows (6): first two carry -W_rel/dt
    wbig[1058:1060, 0:384] = (-W_ih[:, 1058:1060] / DT_CONST).T
    # ones row: input-side biases
    wbig[1064, 0:384] = b_ih
    wbig[1064, 384:512] = b_dh
    wbig = wbig.astype(f16)

    whht = np.ascontiguousarray(W_hh.T).astype(f16)          # (128, 384)
    k3p = np.zeros((66, H), f32)
    k3p[0:2, 0:H] = W_ih[0:128, 1056:1058].T                 # a -> r gate
    k3p[2, 0:H] = b_hh[0:128]
    k3p[32:34] = W_ih[128:256, 1056:1058].T                  # a -> z gate
    k3p[34] = b_hh[128:256]
    k3p[64:66] = W_ih[256:384, 1056:1058].T                  # a -> n gate
    k3p = k3p.astype(f16)
    # head lhsT variants: variant t scatters W_mu/W_std columns to output
    # rows {t, T+t, 2T+t, 3T+t}
    std_off = ((2 * t_steps + 31) // 32) * 32
    m_head = std_off + 2 * t_steps
    wmsx = np.zeros((H, t_steps, m_head), f32)
    for t in range(t_steps):
        wmsx[:, t, t] = W_mu[0]
        wmsx[:, t, t_steps + t] = W_mu[1]
        wmsx[:, t, std_off + t] = W_std[0]
        wmsx[:, t, std_off + t_steps + t] = W_std[1]
    wmsx = wmsx.reshape(H, t_steps * m_head).astype(f16)
    bhhn = b_hh[256:384].reshape(H, 1).astype(f32)
    bmu48 = np.repeat(b_mu, t_steps).reshape(2 * t_steps, 1).astype(f32)
    bstd48 = 0.5 * np.repeat(b_std, t_steps).reshape(2 * t_steps, 1).astype(f32)

    identh = np.eye(H, dtype=f16)
    # host-side tiny matmul for a0 (0.4 MFLOP)
    a0 = last_obs_state @ W_vel.T + b_vel                    # (B, 2)

    in_maps = []
    for c in range(NCORES):
        sl = slice(c * BC, (c + 1) * BC)
        xt = np.empty((KIN, BC), f32)
        xt[0:MLP] = enc_h_feat[sl].T
        xt[MLP:1056] = z[sl].T
        xt[1056:1058] = sg[sl].T
        xt[1058:1064] = last_obs_state[sl].T
        xt[1064] = 1.0
        a3 = np.empty((t_steps, 3, BC), f32)
        a3[0, 0:2] = a0[sl].T
        for t in range(1, t_steps):
            a3[t, 0:2] = fut_traj[t - 1, sl, 2:4].T
        a3[:, 2] = 1.0
        in_maps.append({
            "xt": xt.astype(f16),
            "wbig": wbig,
            "a3": a3.astype(f16),
            "whht": whht,
            "k3p": k3p,
            "wmsx": wmsx,
            "identh": identh,
            "bhhn": bhhn,
            "bmu48": bmu48,
            "bstd48": bstd48,
        })
    return in_maps


def unpack_outputs(results, t_steps=T):
    mus = np.empty((t_steps, B, 2), np.float32)
    stds = np.empty((t_steps, B, 2), np.float32)
    for c in range(NCORES):
        sl = slice(c * BC, (c + 1) * BC)
        omu = results[c]["omu"].reshape(2, t_steps, BC)
        ostd = results[c]["ostd"].reshape(2, t_steps, BC)
        mus[:, sl, 0] = omu[0]
        mus[:, sl, 1] = omu[1]
        stds[:, sl, 0] = ostd[0]
        stds[:, sl, 1] = ostd[1]
    return mus, stds


def kernel(last_obs_state, enc_h_feat, z, sg, fut_traj,
           W_dh, b_dh, W_vel, b_vel, W_ih, b_ih, W_hh, b_hh,
           W_mu, b_mu, W_std, b_std):
    args = dict(
        last_obs_state=np.asarray(last_obs_state, np.float32),
        enc_h_feat=np.asarray(enc_h_feat, np.float32),
        z=np.asarray(z, np.float32),
        sg=np.asarray(sg, np.float32),
        fut_traj=np.asarray(fut_traj, np.float32),
        W_dh=np.asarray(W_dh, np.float32), b_dh=np.asarray(b_dh, np.float32),
        W_vel=np.asarray(W_vel, np.float32), b_vel=np.asarray(b_vel, np.float32),
        W_ih=np.asarray(W_ih, np.float32), b_ih=np.asarray(b_ih, np.float32),
        W_hh=np.asarray(W_hh, np.float32), b_hh=np.asarray(b_hh, np.float32),
        W_mu=np.asarray(W_mu, np.float32), b_mu=np.asarray(b_mu, np.float32),
        W_std=np.asarray(W_std, np.float32), b_std=np.asarray(b_std, np.float32),
    )
    nc = _get_nc()
    in_maps = make_in_maps(**args)
    res = run_bass_kernel_spmd(nc, in_maps, core_ids=list(range(NCORES)))
    return unpack_outputs(res.results)


# revision 13
# speedup vs baseline: 1.5206x; 1.0163x over previous
"""Trainium2 Bass/Tile kernel for nn_Decoder (GRU decoder with teacher forcing).

Math (per reference):
  zx  = [enc_h_feat, z]                    (B, 1056)
  h0  = zx @ W_dh.T + b_dh                 (B, 128)
  a0  = last_obs @ W_vel.T + b_vel         (B, 2)
  rel = (sg - last_obs[:, :2]) / dt        (B, 2)
  a_t = a0 if t==0 else fut_traj[t-1,:,2:4]
  x_t = [zx, a_t, rel]  -> GRUCell(x_t, h) -> mu_t, std_t

Device strategy (8 cores, batch-sharded, 2048 rows/core), all fp16
operands with fp32 PSUM accumulation:
  - Feature-on-partition, batch-on-free layout; free chunks of 512.
  - Setup: [gi_r|gi_z|gi_n|h0](512 rows) = W_big.T @ XT with K=1065
    host-packed rows [zxT; sgT; loT; ones].  The rel term and all
    input-side biases are folded into W_big on the host.
  - Per step/chunk i=(t,c), software-pipelined one chunk deep so PE's
    in-order stream never waits on the sigmoid->q chain:
      stage A(i):  psum_rz  = Whh_{r,z}@h + K3@[a;1] + I@gi_{r,z}   [PE x6]
                   psum_hn  = Whh_n@h                               [PE x1]
                   rz       = sigmoid(psum_rz)                      [ScalarE]
                   psum_gin = (psum_hn + b_hh_n) * r                [DVE stt,
                              written straight into PSUM: warm-start]
      stage B(i-1): psum_gin += I@gi_n + K2@a                       [PE x2]
                   n  = tanh(psum_gin)                              [ScalarE]
                   d = h - n, e = z*d, h' = n + e    [DVE 2-byte 2x tt ops]
    h' for every step is kept in SBUF (25 x 0.5MB fp16 tiles), so no
    PSUM bank is pinned during the loop: rz/hn/gin pools all run bufs=2
    and PE never stalls on a WAR against the activation reads.
  - Post-loop: per-step head matmuls with a scattered-column lhsT
    accumulate mu/std pre-activations for ALL steps into 4 PSUM tiles;
    mu = Identity(+b_mu), std = Exp(0.5*(.)+0.5*b_std) = sqrt(exp(.)).
  - GPSIMD does no work (its tensor ops are ~3-5x slower than DVE);
    DMAs issue from SP/Activation sequencers (HWDGE path, no engine cost).
Host does only sharding/transposes/weight packing (a0 is a (B,6)@(6,2)
matmul on host, ~0.4 MFLOP, negligible vs the 52 GFLOP kernel).
"""

import numpy as np

import concourse.bass as bass
import concourse.mybir as mybir
import concourse.tile as tile
from concourse import bacc
from concourse.bass_utils import run_bass_kernel_spmd

F32 = mybir.dt.float32
F16 = mybir.dt.float16
AF = mybir.ActivationFunctionType
OP = mybir.AluOpType

B, T, MLP, ZD, H, NS, NP = 16384, 24, 1024, 32, 128, 6, 2
NCORES = 8
BC = B // NCORES            # 2048 rows per core
F = 512                     # free-dim chunk
NF = BC // F                # 4 chunks
KIN = MLP + ZD + NP + NS + 1  # 1065 = zx(1056) + sg(2) + lo(6) + ones(1)
NKC = (KIN + 127) // 128    # 9 K-chunks (8x128 + 41)
DT_CONST = 0.4 * 12


def build_nc(debug=False, t_steps=T):
    HD = F16
    nc = bacc.Bacc("TRN2", target_bir_lowering=False, debug=debug)

    # ---- DRAM I/O ----
    xt_d = nc.dram_tensor("xt", [KIN, BC], F16, kind="ExternalInput").ap()
    wbig_d = nc.dram_tensor("wbig", [KIN, 512], F16, kind="ExternalInput").ap()
    a3_d = nc.dram_tensor("a3", [t_steps, 3, BC], F16, kind="ExternalInput").ap()
    whht_d = nc.dram_tensor("whht", [H, 3 * H], F16, kind="ExternalInput").ap()
    k3p_d = nc.dram_tensor("k3p", [66, H], F16, kind="ExternalInput").ap()
    _std_off = ((2 * t_steps + 31) // 32) * 32
    _m_head = _std_off + 2 * t_steps
    wmsx_d = nc.dram_tensor("wmsx", [H, t_steps * _m_head], F16,
                            kind="ExternalInput").ap()
    id_d = nc.dram_tensor("identh", [H, H], F16, kind="ExternalInput").ap()
    bhhn_d = nc.dram_tensor("bhhn", [H, 1], F32, kind="ExternalInput").ap()
    bmu_d = nc.dram_tensor("bmu48", [2 * t_steps, 1], F32, kind="ExternalInput").ap()
    bstd_d = nc.dram_tensor("bstd48", [2 * t_steps, 1], F32, kind="ExternalInput").ap()
    omu_d = nc.dram_tensor("omu", [2 * t_steps, BC], F32, kind="ExternalOutput").ap()
    ostd_d = nc.dram_tensor("ostd", [2 * t_steps, BC], F32, kind="ExternalOutput").ap()

    with tile.TileContext(nc) as tc:
        with tc.tile_pool(name="persist", bufs=1) as pp:
            # persistent SBUF state
            gi_r = pp.tile([H, BC], HD)
            gi_z = pp.tile([H, BC], HD)
            gi_n = pp.tile([H, BC], HD)
            # h history: one tile per step boundary (h_state[t] = h before
            # step t); separate tiles keep dependency tracking per-step
            hst = [pp.tile([H, BC], HD, name=f"hst{t}") for t in range(t_steps + 1)]
            std_off = ((2 * t_steps + 31) // 32) * 32
            m_head = std_off + 2 * t_steps
            whht_t = pp.tile([H, 3 * H], HD)
            k3p_t = pp.tile([66, H], HD)
            wmsx_t = pp.tile([H, t_steps * m_head], HD)
            bhhn_t = pp.tile([H, 1], F32)
            bmu_t = pp.tile([2 * t_steps, 1], F32)
            bstd_t = pp.tile([2 * t_steps, 1], F32)
            ident = pp.tile([H, H], HD)

            gi_dst = [gi_r, gi_z, gi_n, None]

            # ---- setup: [gi | h0] = W_big.T @ XT  (fp16) ----
            with tc.tile_pool(name="xtp", bufs=1) as xtp, \
                 tc.tile_pool(name="wp", bufs=1) as wp, \
                 tc.tile_pool(name="sps", bufs=4, space="PSUM") as sps:
                # one wide DMA per K-chunk (desc size 4KB): DMA-count, not
                # bytes, dominated the old per-(k,chunk) loads
                xt_tiles = []
                w_tiles = []
                for k in range(NKC):
                    kc = min(128, KIN - 128 * k)
                    xt_tiles.append(xtp.tile([kc, BC], HD, name=f"xt{k}",
                                             tag=f"xt{k}"))
                    w_tiles.append(wp.tile([kc, 512], HD, name=f"w{k}",
                                           tag=f"w{k}"))
                # column(nci)-major DMA issue, one slice-DMA per (k, nci):
                # the nci-outer group loop below consumes column 0 for all
                # four M-blocks (7.7us of PE work) while column 1 lands
                # (3.3us) -- PE only ever waits for the first column
                for k in range(NKC):
                    kc = min(128, KIN - 128 * k)
                    nc.gpsimd.dma_start(w_tiles[k][:],
                                        wbig_d[128 * k:128 * k + kc, :])
                for nci in range(NF):
                    for k in range(NKC):
                        kc = min(128, KIN - 128 * k)
                        _eng = [nc.scalar, nc.sync][k % 2]
                        _eng.dma_start(
                            xt_tiles[k][:, nci * F:(nci + 1) * F],
                            xt_d[128 * k:128 * k + kc, nci * F:(nci + 1) * F])
                # loop/head weights ride the Pool SWDGE queue as well --
                # it is idle during setup and bypasses the shared HWDGE
                nc.gpsimd.dma_start(ident[:], id_d)
                nc.gpsimd.dma_start(whht_t[:], whht_d)
                nc.gpsimd.dma_start(k3p_t[:], k3p_d)
                nc.gpsimd.dma_start(bhhn_t[:], bhhn_d)
                nc.gpsimd.dma_start(wmsx_t[:], wmsx_d)
                nc.gpsimd.dma_start(bmu_t[:], bmu_d)
                nc.gpsimd.dma_start(bstd_t[:], bstd_d)

                for nci in range(NF):
                    for m in range(4):
                        ps = sps.tile([128, F], F32, name="setps", tag="setps")
                        for k in range(NKC):
                            nc.tensor.matmul(
                                ps[:], w_tiles[k][:, 128 * m:128 * (m + 1)],
                                xt_tiles[k][:, nci * F:(nci + 1) * F],
                                start=(k == 0), stop=(k == NKC - 1),
                            )
                        dst = gi_dst[m] if m < 3 else hst[0]
                        nc.vector.tensor_copy(dst[:, nci * F:(nci + 1) * F], ps[:])

            # ---- recurrence, software-pipelined one chunk deep ----
            NI = t_steps * NF
            with tc.tile_pool(name="a3p", bufs=3) as a3p, \
                 tc.tile_pool(name="gp", bufs=3) as gp, \
                 tc.tile_pool(name="prz", bufs=2, space="PSUM") as prz, \
                 tc.tile_pool(name="phn", bufs=2, space="PSUM") as phn, \
                 tc.tile_pool(name="pgin", bufs=2, space="PSUM") as pgin:
                a3_tiles = {}
                live = {}   # flat index -> dict of tiles for stage B
                for i in range(NI + 2):
                    # ---- stage C for flat index i-2 (emitted first so the
                    # h' write clears DVE's in-order queue before stt/q2/d
                    # pile in; the next step's matmuls wait on it) ----
                    if i >= 2:
                        lv = live.pop(i - 2)
                        nc.vector.tensor_tensor(hst[lv["t"] + 1][:, lv["cs"]],
                                                lv["nt"][:], lv["e"][:], op=OP.add)
                    # ---- stage A for flat index i ----
                    if i < NI:
                        t, c = divmod(i, NF)
                        cs = slice(c * F, (c + 1) * F)
                        hs = hst[t][:, cs]
                        if c == 0:
                            a3_t = a3p.tile([66, BC], HD, name="a3t", tag="a3t")
                            nc.sync.dma_start(a3_t[0:3, :], a3_d[t])
                            nc.sync.dma_start(a3_t[32:35, :], a3_d[t])
                            nc.sync.dma_start(a3_t[64:66, :], a3_d[t][0:2])
                            a3_tiles[t] = a3_t
                        a3_t = a3_tiles[t]
                        psum_rz = prz.tile([128, 2 * F], F32, name="psrz", tag="psrz")
                        nc.tensor.matmul(psum_rz[:, 0:F], whht_t[:, 0:H], hs,
                                         start=True, stop=False)
                        nc.tensor.matmul(psum_rz[:, F:2 * F], whht_t[:, H:2 * H], hs,
                                         start=True, stop=False)
                        nc.tensor.matmul(psum_rz[:, 0:F], k3p_t[0:3, :],
                                         a3_t[0:3, cs], start=False, stop=False)
                        nc.tensor.matmul(psum_rz[:, F:2 * F], k3p_t[32:35, :],
                                         a3_t[32:35, cs], start=False, stop=False)
                        nc.tensor.matmul(psum_rz[:, 0:F], ident[:], gi_r[:, cs],
                                         start=False, stop=True)
                        nc.tensor.matmul(psum_rz[:, F:2 * F], ident[:], gi_z[:, cs],
                                         start=False, stop=True)
                        psum_hn = phn.tile([128, F], F32, name="pshn", tag="pshn")
                        nc.tensor.matmul(psum_hn[:], whht_t[:, 2 * H:3 * H], hs,
                                         start=True, stop=True)
                        rz = gp.tile([128, 2 * F], HD, name="rz", tag="rz")
                        nc.scalar.activation(rz[:], psum_rz[:], AF.Sigmoid)
                        q = gp.tile([128, F], HD, name="q", tag="q")
                        nc.vector.scalar_tensor_tensor(
                            q[:], psum_hn[:], bhhn_t[:], rz[:, 0:F],
                            op0=OP.add, op1=OP.mult,
                        )
                        live[i] = dict(t=t, cs=cs, hs=hs, rz=rz, a3=a3_t, q=q)
                    # ---- stage B for flat index i-1 ----
                    if 1 <= i <= NI:
                        lv = live[i - 1]
                        t, cs = lv["t"], lv["cs"]
                        # q2 = q + gi_n on DVE replaces a third PE inject
                        q2 = gp.tile([128, F], HD, name="q2", tag="q2")
                        nc.vector.tensor_tensor(q2[:], lv["q"][:], gi_n[:, cs],
                                                op=OP.add)
                        psum_gin = pgin.tile([128, F], F32, name="psgin",
                                             tag="psgin")
                        nc.tensor.matmul(psum_gin[:], k3p_t[64:66, :],
                                         lv["a3"][64:66, cs], start=True,
                                         stop=False)
                        nc.tensor.matmul(psum_gin[:], ident[:], q2[:],
                                         start=False, stop=True)
                        nt = gp.tile([128, F], HD, name="nt", tag="nt")
                        nc.scalar.activation(nt[:], psum_gin[:], AF.Tanh)
                        d = gp.tile([128, F], HD, name="d", tag="d")
                        nc.vector.tensor_tensor(d[:], lv["hs"], nt[:], op=OP.subtract)
                        # e on the otherwise-idle GPSIMD; h' lands one slot
                        # later so the Pool->DVE hop never stalls DVE's queue
                        e = gp.tile([128, F], HD, name="e", tag="e")
                        nc.gpsimd.tensor_tensor(e[:], lv["rz"][:, F:2 * F], d[:],
                                                op=OP.mult)
                        lv["nt"] = nt
                        lv["e"] = e


            # ---- post-loop heads: c-outer so each chunk's finalize
            # (ACT straight from PSUM + out-DMA) overlaps the next chunk's
            # 24-matmul accumulation stream ----
            with tc.tile_pool(name="phd", bufs=2, space="PSUM") as phd, \
                 tc.tile_pool(name="fin", bufs=2) as fin:
                for c in range(NF):
                    cs = slice(c * F, (c + 1) * F)
                    ps = phd.tile([m_head, F], F32, name="pshd", tag="pshd")
                    for t in range(t_steps):
                        nc.tensor.matmul(
                            ps[:],
                            wmsx_t[:, t * m_head:(t + 1) * m_head],
                            hst[t + 1][:, cs],
                            start=(t == 0), stop=(t == t_steps - 1),
                        )
                    mu_c = fin.tile([2 * t_steps, F], F32, name="muc", tag="muc")
                    std_c = fin.tile([2 * t_steps, F], F32, name="stdc", tag="stdc")
                    nc.scalar.activation(mu_c[:], ps[0:2 * t_steps, :],
                                         AF.Identity, bias=bmu_t[:])
                    nc.scalar.activation(std_c[:],
                                         ps[std_off:std_off + 2 * t_steps, :],
                                         AF.Exp, bias=bstd_t[:], scale=0.5)
                    nc.sync.dma_start(omu_d[:, cs], mu_c[:])
                    nc.sync.dma_start(ostd_d[:, cs], std_c[:])

    nc.compile()
    return nc


_NC_CACHE = {}


def _get_nc(debug=False):
    if "nc" not in _NC_CACHE:
        _NC_CACHE["nc"] = build_nc(debug=debug)
    return _NC_CACHE["nc"]


def make_in_maps(last_obs_state, enc_h_feat, z, sg, fut_traj,
                 W_dh, b_dh, W_vel, b_vel, W_ih, b_ih, W_hh, b_hh,
                 W_mu, b_mu, W_std, b_std, t_steps=T):
    f32 = np.float32
    f16 = np.float16

    # ---- weight packing (core-independent) ----
    # W_big: (KIN, 512) ; out cols = [gi_r, gi_z, gi_n, h0]
    wbig = np.zeros((KIN, 512), f32)
    wbig[0:1056, 0:384] = W_ih[:, 0:1056].T
    wbig[0:1056, 384:512] = W_dh.T
    # sg rows: rel = (sg - lo[:, :2])/dt feeds W_ih[:, 1058:1060]
    wbig[1056:1058, 0:384] = (W_ih[:, 1058:1060] / DT_CONST).T
    # lo rows (6): first two carry -W_rel/dt
    wbig[1058:1060, 0:384] = (-W_ih[:, 1058:1060] / DT_CONST).T
    # ones row: input-side biases
    wbig[1064, 0:384] = b_ih
    wbig[1064, 384:512] = b_dh
    wbig = wbig.astype(f16)

    whht = np.ascontiguousarray(W_hh.T).astype(f16)          # (128, 384)
    k3p = np.zeros((66, H), f32)
    k3p[0:2, 0:H] = W_ih[0:128, 1056:1058].T                 # a -> r gate
    k3p[2, 0:H] = b_hh[0:128]
    k3p[32:34] = W_ih[128:256, 1056:1058].T                  # a -> z gate
    k3p[34] = b_hh[128:256]
    k3p[64:66] = W_ih[256:384, 1056:1058].T                  # a -> n gate
    k3p = k3p.astype(f16)
    # head lhsT variants: variant t scatters W_mu/W_std columns to output
    # rows {t, T+t, 2T+t, 3T+t}
    std_off = ((2 * t_steps + 31) // 32) * 32
    m_head = std_off + 2 * t_steps
    wmsx = np.zeros((H, t_steps, m_head), f32)
    for t in range(t_steps):
        wmsx[:, t, t] = W_mu[0]
        wmsx[:, t, t_steps + t] = W_mu[1]
        wmsx[:, t, std_off + t] = W_std[0]
        wmsx[:, t, std_off + t_steps + t] = W_std[1]
    wmsx = wmsx.reshape(H, t_steps * m_head).astype(f16)
    bhhn = b_hh[256:384].reshape(H, 1).astype(f32)
    bmu48 = np.repeat(b_mu, t_steps).reshape(2 * t_steps, 1).astype(f32)
    bstd48 = 0.5 * np.repeat(b_std, t_steps).reshape(2 * t_steps, 1).astype(f32)

    identh = np.eye(H, dtype=f16)
    # host-side tiny matmul for a0 (0.4 MFLOP)
    a0 = last_obs_state @ W_vel.T + b_vel                    # (B, 2)

    in_maps = []
    for c in range(NCORES):
        sl = slice(c * BC, (c + 1) * BC)
        xt = np.empty((KIN, BC), f32)
        xt[0:MLP] = enc_h_feat[sl].T
        xt[MLP:1056] = z[sl].T
        xt[1056:1058] = sg[sl].T
        xt[1058:1064] = last_obs_state[sl].T
        xt[1064] = 1.0
        a3 = np.empty((t_steps, 3, BC), f32)
        a3[0, 0:2] = a0[sl].T
        for t in range(1, t_steps):
            a3[t, 0:2] = fut_traj[t - 1, sl, 2:4].T
        a3[:, 2] = 1.0
        in_maps.append({
            "xt": xt.astype(f16),
            "wbig": wbig,
            "a3": a3.astype(f16),
            "whht": whht,
            "k3p": k3p,
            "wmsx": wmsx,
            "identh": identh,
            "bhhn": bhhn,
            "bmu48": bmu48,
            "bstd48": bstd48,
        })
    return in_maps


def unpack_outputs(results, t_steps=T):
    mus = np.empty((t_steps, B, 2), np.float32)
    stds = np.empty((t_steps, B, 2), np.float32)
    for c in range(NCORES):
        sl = slice(c * BC, (c + 1) * BC)
        omu = results[c]["omu"].reshape(2, t_steps, BC)
        ostd = results[c]["ostd"].reshape(2, t_steps, BC)
        mus[:, sl, 0] = omu[0]
        mus[:, sl, 1] = omu[1]
        stds[:, sl, 0] = ostd[0]
        stds[:, sl, 1] = ostd[1]
    return mus, stds


def kernel(last_obs_state, enc_h_feat, z, sg, fut_traj,
           W_dh, b_dh, W_vel, b_vel, W_ih, b_ih, W_hh, b_hh,
           W_mu, b_mu, W_std, b_std):
    args = dict(
        last_obs_state=np.asarray(last_obs_state, np.float32),
        enc_h_feat=np.asarray(enc_h_feat, np.float32),
        z=np.asarray(z, np.float32),
        sg=np.asarray(sg, np.float32),
        fut_traj=np.asarray(fut_traj, np.float32),
        W_dh=np.asarray(W_dh, np.float32), b_dh=np.asarray(b_dh, np.float32),
        W_vel=np.asarray(W_vel, np.float32), b_vel=np.asarray(b_vel, np.float32),
        W_ih=np.asarray(W_ih, np.float32), b_ih=np.asarray(b_ih, np.float32),
        W_hh=np.asarray(W_hh, np.float32), b_hh=np.asarray(b_hh, np.float32),
        W_mu=np.asarray(W_mu, np.float32), b_mu=np.asarray(b_mu, np.float32),
        W_std=np.asarray(W_std, np.float32), b_std=np.asarray(b_std, np.float32),
    )
    nc = _get_nc()
    in_maps = make_in_maps(**args)
    res = run_bass_kernel_spmd(nc, in_maps, core_ids=list(range(NCORES)))
    return unpack_outputs(res.results)


# revision 22
# speedup vs baseline: 1.5242x; 1.0024x over previous
"""Trainium2 Bass/Tile kernel for nn_Decoder (GRU decoder with teacher forcing).

Math (per reference):
  zx  = [enc_h_feat, z]                    (B, 1056)
  h0  = zx @ W_dh.T + b_dh                 (B, 128)
  a0  = last_obs @ W_vel.T + b_vel         (B, 2)
  rel = (sg - last_obs[:, :2]) / dt        (B, 2)
  a_t = a0 if t==0 else fut_traj[t-1,:,2:4]
  x_t = [zx, a_t, rel]  -> GRUCell(x_t, h) -> mu_t, std_t

Device strategy (8 cores, batch-sharded, 2048 rows/core), all fp16
operands with fp32 PSUM accumulation:
  - Feature-on-partition, batch-on-free layout; free chunks of 512.
  - Setup: [gi_r|gi_z|gi_n|h0](512 rows) = W_big.T @ XT with K=1065
    host-packed rows [zxT; sgT; loT; ones].  The rel term and all
    input-side biases are folded into W_big on the host.
  - Per step/chunk i=(t,c), software-pipelined one chunk deep so PE's
    in-order stream never waits on the sigmoid->q chain:
      stage A(i):  psum_rz  = Whh_{r,z}@h + K3@[a;1] + I@gi_{r,z}   [PE x6]
                   psum_hn  = Whh_n@h                               [PE x1]
                   rz       = sigmoid(psum_rz)                      [ScalarE]
                   psum_gin = (psum_hn + b_hh_n) * r                [DVE stt,
                              written straight into PSUM: warm-start]
      stage B(i-1): psum_gin += I@gi_n + K2@a                       [PE x2]
                   n  = tanh(psum_gin)                              [ScalarE]
                   d = h - n, e = z*d, h' = n + e    [DVE 2-byte 2x tt ops]
    h' for every step is kept in SBUF (25 x 0.5MB fp16 tiles), so no
    PSUM bank is pinned during the loop: rz/hn/gin pools all run bufs=2
    and PE never stalls on a WAR against the activation reads.
  - Post-loop: per-step head matmuls with a scattered-column lhsT
    accumulate mu/std pre-activations for ALL steps into 4 PSUM tiles;
    mu = Identity(+b_mu), std = Exp(0.5*(.)+0.5*b_std) = sqrt(exp(.)).
  - GPSIMD does no work (its tensor ops are ~3-5x slower than DVE);
    DMAs issue from SP/Activation sequencers (HWDGE path, no engine cost).
Host does only sharding/transposes/weight packing (a0 is a (B,6)@(6,2)
matmul on host, ~0.4 MFLOP, negligible vs the 52 GFLOP kernel).
"""

import numpy as np

import concourse.bass as bass
import concourse.mybir as mybir
import concourse.tile as tile
from concourse import bacc
from concourse.bass_utils import run_bass_kernel_spmd

F32 = mybir.dt.float32
F16 = mybir.dt.float16
AF = mybir.ActivationFunctionType
OP = mybir.AluOpType

B, T, MLP, ZD, H, NS, NP = 16384, 24, 1024, 32, 128, 6, 2
NCORES = 8
BC = B // NCORES            # 2048 rows per core
F = 512                     # free-dim chunk
NF = BC // F                # 4 chunks
KIN = MLP + ZD + NP + NS + 1  # 1065 = zx(1056) + sg(2) + lo(6) + ones(1)
NKC = (KIN + 127) // 128    # 9 K-chunks (8x128 + 41)
DT_CONST = 0.4 * 12


def build_nc(debug=False, t_steps=T):
    HD = F16
    nc = bacc.Bacc("TRN2", target_bir_lowering=False, debug=debug)

    # ---- DRAM I/O ----
    xt_d = nc.dram_tensor("xt", [KIN, BC], F16, kind="ExternalInput").ap()
    wbig_d = nc.dram_tensor("wbig", [KIN, 512], F16, kind="ExternalInput").ap()
    a3_d = nc.dram_tensor("a3", [t_steps, 3, BC], F16, kind="ExternalInput").ap()
    whht_d = nc.dram_tensor("whht", [H, 3 * H], F16, kind="ExternalInput").ap()
    k3p_d = nc.dram_tensor("k3p", [66, H], F16, kind="ExternalInput").ap()
    _std_off = ((2 * t_steps + 31) // 32) * 32
    _m_head = _std_off + 2 * t_steps
    wmsx_d = nc.dram_tensor("wmsx", [H, t_steps * _m_head], F16,
                            kind="ExternalInput").ap()
    id_d = nc.dram_tensor("identh", [H, H], F16, kind="ExternalInput").ap()
    bhhn_d = nc.dram_tensor("bhhn", [H, 1], F32, kind="ExternalInput").ap()
    bmu_d = nc.dram_tensor("bmu48", [2 * t_steps, 1], F32, kind="ExternalInput").ap()
    bstd_d = nc.dram_tensor("bstd48", [2 * t_steps, 1], F32, kind="ExternalInput").ap()
    omu_d = nc.dram_tensor("omu", [2 * t_steps, BC], F32, kind="ExternalOutput").ap()
    ostd_d = nc.dram_tensor("ostd", [2 * t_steps, BC], F32, kind="ExternalOutput").ap()

    with tile.TileContext(nc) as tc:
        with tc.tile_pool(name="persist", bufs=1) as pp:
            # persistent SBUF state
            gi_r = pp.tile([H, BC], HD)
            gi_z = pp.tile([H, BC], HD)
            gi_n = pp.tile([H, BC], HD)
            # h history: one tile per step boundary (h_state[t] = h before
            # step t); separate tiles keep dependency tracking per-step
            hst = [pp.tile([H, BC], HD, name=f"hst{t}") for t in range(t_steps + 1)]
            std_off = ((2 * t_steps + 31) // 32) * 32
            m_head = std_off + 2 * t_steps
            whht_t = pp.tile([H, 3 * H], HD)
            k3p_t = pp.tile([66, H], HD)
            wmsx_t = pp.tile([H, t_steps * m_head], HD)
            bhhn_t = pp.tile([H, 1], F32)
            bmu_t = pp.tile([2 * t_steps, 1], F32)
            bstd_t = pp.tile([2 * t_steps, 1], F32)
            ident = pp.tile([H, H], HD)

            gi_dst = [gi_r, gi_z, gi_n, None]

            # ---- setup: [gi | h0] = W_big.T @ XT  (fp16) ----
            with tc.tile_pool(name="xtp", bufs=1) as xtp, \
                 tc.tile_pool(name="wp", bufs=1) as wp, \
                 tc.tile_pool(name="sps", bufs=4, space="PSUM") as sps:
                # one wide DMA per K-chunk (desc size 4KB): DMA-count, not
                # bytes, dominated the old per-(k,chunk) loads
                xt_tiles = []
                w_tiles = []
                for k in range(NKC):
                    kc = min(128, KIN - 128 * k)
                    xt_tiles.append(xtp.tile([kc, BC], HD, name=f"xt{k}",
                                             tag=f"xt{k}"))
                    w_tiles.append(wp.tile([kc, 512], HD, name=f"w{k}",
                                           tag=f"w{k}"))
                # column(nci)-major DMA issue, one slice-DMA per (k, nci):
                # the nci-outer group loop below consumes column 0 for all
                # four M-blocks (7.7us of PE work) while column 1 lands
                # (3.3us) -- PE only ever waits for the first column
                for k in range(NKC):
                    kc = min(128, KIN - 128 * k)
                    # w0 via the HWDGE path in parallel with Pool's SWDGE
                    # chain so the very first matmul isn't gated on Pool
                    _eng = nc.sync if k == 0 else nc.gpsimd
                    _eng.dma_start(w_tiles[k][:],
                                   wbig_d[128 * k:128 * k + kc, :])
                for nci in range(NF):
                    for k in range(NKC):
                        kc = min(128, KIN - 128 * k)
                        _eng = [nc.scalar, nc.sync][k % 2]
                        _eng.dma_start(
                            xt_tiles[k][:, nci * F:(nci + 1) * F],
                            xt_d[128 * k:128 * k + kc, nci * F:(nci + 1) * F])
                # loop/head weights ride the Pool SWDGE queue as well --
                # it is idle during setup and bypasses the shared HWDGE
                nc.gpsimd.dma_start(ident[:], id_d)
                nc.gpsimd.dma_start(whht_t[:], whht_d)
                nc.gpsimd.dma_start(k3p_t[:], k3p_d)
                nc.gpsimd.dma_start(bhhn_t[:], bhhn_d)
                nc.gpsimd.dma_start(wmsx_t[:], wmsx_d)
                nc.gpsimd.dma_start(bmu_t[:], bmu_d)
                nc.gpsimd.dma_start(bstd_t[:], bstd_d)

                for nci in range(NF):
                    for m in range(4):
                        ps = sps.tile([128, F], F32, name="setps", tag="setps")
                        for k in range(NKC):
                            nc.tensor.matmul(
                                ps[:], w_tiles[k][:, 128 * m:128 * (m + 1)],
                                xt_tiles[k][:, nci * F:(nci + 1) * F],
                                start=(k == 0), stop=(k == NKC - 1),
                            )
                        dst = gi_dst[m] if m < 3 else hst[0]
                        nc.vector.tensor_copy(dst[:, nci * F:(nci + 1) * F], ps[:])

            # ---- recurrence, software-pipelined one chunk deep ----
            NI = t_steps * NF
            with tc.tile_pool(name="a3p", bufs=3) as a3p, \
                 tc.tile_pool(name="gp", bufs=3) as gp, \
                 tc.tile_pool(name="prz", bufs=2, space="PSUM") as prz, \
                 tc.tile_pool(name="phn", bufs=2, space="PSUM") as phn, \
                 tc.tile_pool(name="pgin", bufs=2, space="PSUM") as pgin:
                a3_tiles = {}
                live = {}   # flat index -> dict of tiles for stage B
                for i in range(NI + 2):
                    # ---- stage C for flat index i-2 (emitted first so the
                    # h' write clears DVE's in-order queue before stt/q2/d
                    # pile in; the next step's matmuls wait on it) ----
                    if i >= 2:
                        lv = live.pop(i - 2)
                        nc.vector.tensor_tensor(hst[lv["t"] + 1][:, lv["cs"]],
                                                lv["nt"][:], lv["e"][:], op=OP.add)
                    # ---- stage A for flat index i ----
                    if i < NI:
                        t, c = divmod(i, NF)
                        cs = slice(c * F, (c + 1) * F)
                        hs = hst[t][:, cs]
                        if c == 0:
                            a3_t = a3p.tile([66, BC], HD, name="a3t", tag="a3t")
                            nc.sync.dma_start(a3_t[0:3, :], a3_d[t])
                            nc.sync.dma_start(a3_t[32:35, :], a3_d[t])
                            nc.sync.dma_start(a3_t[64:66, :], a3_d[t][0:2])
                            a3_tiles[t] = a3_t
                        a3_t = a3_tiles[t]
                        psum_rz = prz.tile([128, 2 * F], F32, name="psrz", tag="psrz")
                        nc.tensor.matmul(psum_rz[:, 0:F], whht_t[:, 0:H], hs,
                                         start=True, stop=False)
                        nc.tensor.matmul(psum_rz[:, F:2 * F], whht_t[:, H:2 * H], hs,
                                         start=True, stop=False)
                        nc.tensor.matmul(psum_rz[:, 0:F], k3p_t[0:3, :],
                                         a3_t[0:3, cs], start=False, stop=False)
                        nc.tensor.matmul(psum_rz[:, F:2 * F], k3p_t[32:35, :],
                                         a3_t[32:35, cs], start=False, stop=False)
                        nc.tensor.matmul(psum_rz[:, 0:F], ident[:], gi_r[:, cs],
                                         start=False, stop=True)
                        nc.tensor.matmul(psum_rz[:, F:2 * F], ident[:], gi_z[:, cs],
                                         start=False, stop=True)
                        psum_hn = phn.tile([128, F], F32, name="pshn", tag="pshn")
                        nc.tensor.matmul(psum_hn[:], whht_t[:, 2 * H:3 * H], hs,
                                         start=True, stop=True)
                        rz = gp.tile([128, 2 * F], HD, name="rz", tag="rz")
                        nc.scalar.activation(rz[:], psum_rz[:], AF.Sigmoid)
                        q = gp.tile([128, F], HD, name="q", tag="q")
                        nc.vector.scalar_tensor_tensor(
                            q[:], psum_hn[:], bhhn_t[:], rz[:, 0:F],
                            op0=OP.add, op1=OP.mult,
                        )
                        live[i] = dict(t=t, cs=cs, hs=hs, rz=rz, a3=a3_t, q=q)
                    # ---- stage B for flat index i-1 ----
                    if 1 <= i <= NI:
                        lv = live[i - 1]
                        t, cs = lv["t"], lv["cs"]
                        # q2 = q + gi_n on DVE replaces a third PE inject
                        q2 = gp.tile([128, F], HD, name="q2", tag="q2")
                        nc.vector.tensor_tensor(q2[:], lv["q"][:], gi_n[:, cs],
                                                op=OP.add)
                        psum_gin = pgin.tile([128, F], F32, name="psgin",
                                             tag="psgin")
                        nc.tensor.matmul(psum_gin[:], k3p_t[64:66, :],
                                         lv["a3"][64:66, cs], start=True,
                                         stop=False)
                        nc.tensor.matmul(psum_gin[:], ident[:], q2[:],
                                         start=False, stop=True)
                        nt = gp.tile([128, F], HD, name="nt", tag="nt")
                        nc.scalar.activation(nt[:], psum_gin[:], AF.Tanh)
                        d = gp.tile([128, F], HD, name="d", tag="d")
                        nc.vector.tensor_tensor(d[:], lv["hs"], nt[:], op=OP.subtract)
                        # e on the otherwise-idle GPSIMD; h' lands one slot
                        # later so the Pool->DVE hop never stalls DVE's queue
                        e = gp.tile([128, F], HD, name="e", tag="e")
                        nc.gpsimd.tensor_tensor(e[:], lv["rz"][:, F:2 * F], d[:],
                                                op=OP.mult)
                        lv["nt"] = nt
                        lv["e"] = e


            # ---- post-loop heads: c-outer so each chunk's finalize
            # (ACT straight from PSUM + out-DMA) overlaps the next chunk's
            # 24-matmul accumulation stream ----
            with tc.tile_pool(name="phd", bufs=2, space="PSUM") as phd, \
                 tc.tile_pool(name="fin", bufs=2) as fin:
                for c in range(NF):
                    cs = slice(c * F, (c + 1) * F)
                    ps = phd.tile([m_head, F], F32, name="pshd", tag="pshd")
                    for t in range(t_steps):
                        nc.tensor.matmul(
                            ps[:],
                            wmsx_t[:, t * m_head:(t + 1) * m_head],
                            hst[t + 1][:, cs],
                            start=(t == 0), stop=(t == t_steps - 1),
                        )
                    mu_c = fin.tile([2 * t_steps, F], F32, name="muc", tag="muc")
                    std_c = fin.tile([2 * t_steps, F], F32, name="stdc", tag="stdc")
                    nc.scalar.activation(mu_c[:], ps[0:2 * t_steps, :],
                                         AF.Identity, bias=bmu_t[:])
                    nc.sync.dma_start(omu_d[:, cs], mu_c[:])
                    nc.scalar.activation(std_c[:],
                                         ps[std_off:std_off + 2 * t_steps, :],
                                         AF.Exp, bias=bstd_t[:], scale=0.5)
                    nc.sync.dma_start(ostd_d[:, cs], std_c[:])

    nc.compile()
    return nc


_NC_CACHE = {}


def _get_nc(debug=False):
    if "nc" not in _NC_CACHE:
        _NC_CACHE["nc"] = build_nc(debug=debug)
    return _NC_CACHE["nc"]


def make_in_maps(last_obs_state, enc_h_feat, z, sg, fut_traj,
                 W_dh, b_dh, W_vel, b_vel, W_ih, b_ih, W_hh, b_hh,
                 W_mu, b_mu, W_std, b_std, t_steps=T):
    f32 = np.float32
    f16 = np.float16

    # ---- weight packing (core-independent) ----
    # W_big: (KIN, 512) ; out cols = [gi_r, gi_z, gi_n, h0]
    wbig = np.zeros((KIN, 512), f32)
    wbig[0:1056, 0:384] = W_ih[:, 0:1056].T
    wbig[0:1056, 384:512] = W_dh.T
    # sg rows: rel = (sg - lo[:, :2])/dt feeds W_ih[:, 1058:1060]
    wbig[1056:1058, 0:384] = (W_ih[:, 1058:1060] / DT_CONST).T
    # lo rows (6): first two carry -W_rel/dt
    wbig[1058:1060, 0:384] = (-W_ih[:, 1058:1060] / DT_CONST).T
    # ones row: input-side biases
    wbig[1064, 0:384] = b_ih
    wbig[1064, 384:512] = b_dh
    wbig = wbig.astype(f16)

    whht = np.ascontiguousarray(W_hh.T).astype(f16)          # (128, 384)
    k3p = np.zeros((66, H), f32)
    k3p[0:2, 0:H] = W_ih[0:128, 1056:1058].T                 # a -> r gate
    k3p[2, 0:H] = b_hh[0:128]
    k3p[32:34] = W_ih[128:256, 1056:1058].T                  # a -> z gate
    k3p[34] = b_hh[128:256]
    k3p[64:66] = W_ih[256:384, 1056:1058].T                  # a -> n gate
    k3p = k3p.astype(f16)
    # head lhsT variants: variant t scatters W_mu/W_std columns to output
    # rows {t, T+t, 2T+t, 3T+t}
    std_off = ((2 * t_steps + 31) // 32) * 32
    m_head = std_off + 2 * t_steps
    wmsx = np.zeros((H, t_steps, m_head), f32)
    for t in range(t_steps):
        wmsx[:, t, t] = W_mu[0]
        wmsx[:, t, t_steps + t] = W_mu[1]
        wmsx[:, t, std_off + t] = W_std[0]
        wmsx[:, t, std_off + t_steps + t] = W_std[1]
    wmsx = wmsx.reshape(H, t_steps * m_head).astype(f16)
    bhhn = b_hh[256:384].reshape(H, 1).astype(f32)
    bmu48 = np.repeat(b_mu, t_steps).reshape(2 * t_steps, 1).astype(f32)
    bstd48 = 0.5 * np.repeat(b_std, t_steps).reshape(2 * t_steps, 1).astype(f32)

    identh = np.eye(H, dtype=f16)
    # host-side tiny matmul for a0 (0.4 MFLOP)
    a0 = last_obs_state @ W_vel.T + b_vel                    # (B, 2)

    in_maps = []
    for c in range(NCORES):
        sl = slice(c * BC, (c + 1) * BC)
        xt = np.empty((KIN, BC), f32)
        xt[0:MLP] = enc_h_feat[sl].T
        xt[MLP:1056] = z[sl].T
        xt[1056:1058] = sg[sl].T
        xt[1058:1064] = last_obs_state[sl].T
        xt[1064] = 1.0
        a3 = np.empty((t_steps, 3, BC), f32)
        a3[0, 0:2] = a0[sl].T
        for t in range(1, t_steps):
            a3[t, 0:2] = fut_traj[t - 1, sl, 2:4].T
        a3[:, 2] = 1.0
        in_maps.append({
            "xt": xt.astype(f16),
            "wbig": wbig,
            "a3": a3.astype(f16),
            "whht": whht,
            "k3p": k3p,
            "wmsx": wmsx,
            "identh": identh,
            "bhhn": bhhn,
            "bmu48": bmu48,
            "bstd48": bstd48,
        })
    return in_maps


def unpack_outputs(results, t_steps=T):
    mus = np.empty((t_steps, B, 2), np.float32)
    stds = np.empty((t_steps, B, 2), np.float32)
    for c in range(NCORES):
        sl = slice(c * BC, (c + 1) * BC)
        omu = results[c]["omu"].reshape(2, t_steps, BC)
        ostd = results[c]["ostd"].reshape(2, t_steps, BC)
        mus[:, sl, 0] = omu[0]
        mus[:, sl, 1] = omu[1]
        stds[:, sl, 0] = ostd[0]
        stds[:, sl, 1] = ostd[1]
    return mus, stds


def kernel(last_obs_state, enc_h_feat, z, sg, fut_traj,
           W_dh, b_dh, W_vel, b_vel, W_ih, b_ih, W_hh, b_hh,
           W_mu, b_mu, W_std, b_std):
    args = dict(
        last_obs_state=np.asarray(last_obs_state, np.float32),
        enc_h_feat=np.asarray(enc_h_feat, np.float32),
        z=np.asarray(z, np.float32),
        sg=np.asarray(sg, np.float32),
        fut_traj=np.asarray(fut_traj, np.float32),
        W_dh=np.asarray(W_dh, np.float32), b_dh=np.asarray(b_dh, np.float32),
        W_vel=np.asarray(W_vel, np.float32), b_vel=np.asarray(b_vel, np.float32),
        W_ih=np.asarray(W_ih, np.float32), b_ih=np.asarray(b_ih, np.float32),
        W_hh=np.asarray(W_hh, np.float32), b_hh=np.asarray(b_hh, np.float32),
        W_mu=np.asarray(W_mu, np.float32), b_mu=np.asarray(b_mu, np.float32),
        W_std=np.asarray(W_std, np.float32), b_std=np.asarray(b_std, np.float32),
    )
    nc = _get_nc()
    in_maps = make_in_maps(**args)
    res = run_bass_kernel_spmd(nc, in_maps, core_ids=list(range(NCORES)))
    return unpack_outputs(res.results)
